# revision 2
# baseline (speedup 1.0000x reference)
"""Trainium2 Bass kernel for the Baller2Vec-style transformer encoder, v2.

Data parallel over batch (B=128) across 8 NeuronCores (16 per core, 8 pairs
of 2).  Feature-major activations.  The residual stream stores the
*unaffined* LayerNorm output (X-hat); each LN's per-feature gain/bias is
folded host-side into the adjacent weights and bias rows, removing three
DVE passes per LN.  QKV+V projections run as fp8(e4m3) DoubleRow matmuls
(weights hi+lo 2-term, activations hi) at 2x PE throughput; the rest is
bf16 with fp32 PSUM.  Sublayer biases enter PSUM via rank-1 matmuls;
softmax row-sum reciprocals are batched 3 heads per instruction; exp is
batched over 2-bank PSUM score tiles (4 segments per Activation op); FFN
relu runs on the otherwise idle Pool engine.
"""
import sys
import numpy as np

sys.path.insert(0, '/opt/trn_rl_repo')

import ml_dtypes
import concourse.bass as bass
import concourse.mybir as mybir
from concourse import tile
from concourse.bass_utils import run_bass_kernel_spmd
from concourse.vector_clock import ScopedClock

# ---------------------------------------------------------------- constants
HMAP = {0: (0, 0), 1: (0, 1), 2: (1, 0), 3: (1, 1), 4: (2, 0), 5: (2, 1),
        6: (0, 2), 7: (1, 2)}
BC_TILES = {0: [0], 1: [1], 2: [2], 3: [0, 1]}
P, T, E, D, H, F, L, V, C = 10, 20, 64, 512, 8, 2048, 6, 512, 9
B = 128
S = (P + 2) * T          # 240
HD = D // H
SCALE = float(np.sqrt(D))
HSCALE = 1.0 / float(np.sqrt(HD))
NCORES = 8
BPC = B // NCORES
NPAIR = BPC // 2
PC = 2 * S               # 480
NC_CH = D // 128
NF_CH = F // 128
TK = 120
EPS = 1e-5
SX = 16.0                # fp8 activation scale

bf16 = mybir.dt.bfloat16
f32 = mybir.dt.float32
fp8 = mybir.dt.float8e4
AF = mybir.ActivationFunctionType
ALU = mybir.AluOpType
PM = mybir.MatmulPerfMode
e4np = ml_dtypes.float8_e4m3fn


def _mask_np():
    m = np.full((S, S), -np.inf, dtype=np.float32)
    bs, cs = P * T, P * T + T
    for s in range(T):
        a, b_ = P * s, P * s + P
        m[a:b_, :b_] = 0.0
        m[a:b_, bs:bs + s + 1] = 0.0
        m[a:b_, cs:cs + s + 1] = 0.0
        for r in (bs + s, cs + s):
            m[r, :b_] = 0.0
            m[r, bs:bs + s + 1] = 0.0
            m[r, cs:cs + s + 1] = 0.0
    return m


# ---------------------------------------------------------------- tile patch
MAX_WAITS_PER_INST = 1


def _patched_drain_and_barrier(self, tick_clock, wait_clock):
    nc = self.nc
    probe = nc.sync.nop(nofuse=True)
    wait_clock.add_sem_waits(probe.ins, ScopedClock({None: tick_clock.global_clock}))
    si = probe.ins.sync_info
    waits = list(si.on_wait) if si is not None else []
    K = MAX_WAITS_PER_INST
    probe.ins.sync_info = mybir.SyncInfo(on_wait=waits[:K], on_update=[])
    for j in range(K, len(waits), K):
        w = nc.sync.nop(nofuse=True)
        w.ins.sync_info = mybir.SyncInfo(on_wait=waits[j:j + K], on_update=[])
    nc.sync.drain()
    nc.all_engine_barrier()
    popped = nc._tile_sem_poison_stack.pop()
    assert popped is self._sem_poison
    nc.clear_and_free_semaphores(list(self.sems.allocated().values()))
    nc.all_engine_barrier()


tile.TileContext._drain_and_barrier = _patched_drain_and_barrier


def _split_waits(nc, K=1):
    wsid = [0]
    for bb in nc.m.functions[0].blocks:
        out = []
        changed = False
        for inst in bb.instructions:
            si = inst.sync_info
            waits = list(si.on_wait) if si is not None else []
            if len(waits) > K:
                changed = True
                extra = waits[:-K]
                for j in range(0, len(extra), K):
                    nop = mybir.InstNoOp(name=f"WSNOP-{wsid[0]}")
                    wsid[0] += 1
                    nop.engine = inst.engine
                    nop.sync_info = mybir.SyncInfo(on_wait=extra[j:j + K],
                                                   on_update=[])
                    out.append(nop)
                inst.sync_info = mybir.SyncInfo(on_wait=waits[-K:],
                                                on_update=list(si.on_update))
            out.append(inst)
        if changed:
            bb.instructions = out


# ---------------------------------------------------------------- builder
def build(qdescale=(1.0,) * L, n_layers=L, n_pairs=NPAIR):
    """qdescale[l]: immediate multiplier applied when copying the fp8
    QKV/V psum back to bf16 (1/(sx*sw)); qdescale[0] unused."""
    nc = bass.Bass("TRN2", target_bir_lowering=False)
    dt_in = {
        'pinT': ([E + 2, BPC * T * P], bf16),
        'ballT': ([E + 2, BPC * T], bf16),
        'maskb': ([TK, 2, S], bf16),
        'pw1': ([E + 2, 128], bf16),
        'pw2': ([128, NC_CH * 128], bf16),
        'bw1': ([E + 2, 128], bf16),
        'bw2': ([128, NC_CH * 128], bf16),
        'pb1': ([128, 1], f32),
        'pb2s': ([128, NC_CH], f32),
        'bb1': ([128, 1], f32),
        'bb2s': ([128, NC_CH], f32),
        'wq0': ([128, NC_CH, 3 * D], bf16),
        'wq8': ([L, 128, 2, 2, 2, 3 * D], fp8),
        'wo': ([L, 128, NC_CH, D], bf16),
        'wf1': ([L, 128, NC_CH, F], bf16),
        'wf2': ([L, 128, NF_CH, D], bf16),
        'brows': ([L, 1, 2 * NC_CH, 128], bf16),
        'cvec': ([128, L, 32], f32),
        'selcf': ([65, 3, NC_CH * 128], bf16),
        'onesbf': ([128, 1], bf16),
        'onesd': ([128, 1], bf16),
        'onesrow': ([1, PC], bf16),
        'clsw': ([128, NC_CH, C], bf16),
        'clsb': ([C, 1], f32),
        'clse': ([128, NC_CH], f32),
    }
    dins = {k: nc.dram_tensor(k, sh, dt, kind="ExternalInput")
            for k, (sh, dt) in dt_in.items()}
    dout = nc.dram_tensor("out", [C, BPC], f32, kind="ExternalOutput")

    with tile.TileContext(nc) as tc:
        _body(nc, tc, dins, dout, qdescale, n_layers, n_pairs)
    _split_waits(nc)
    return nc


def _body(nc, tc, dins, dout, qdescale, n_layers, n_pairs):
    import contextlib
    ctx = contextlib.ExitStack()
    with ctx:
        persist = ctx.enter_context(tc.tile_pool(name="persist", bufs=1))
        X = persist.tile([128, NC_CH, BPC * S], bf16)     # residual (X-hat)
        XQ8 = persist.tile([128, NC_CH, BPC * S], fp8)    # fp8 shadow for qkv

        maskb = persist.tile([TK, 2, S], bf16)
        nc.sync.dma_start(maskb[:], dins['maskb'][:])
        cvec = persist.tile([128, L, 32], f32)
        nc.sync.dma_start(cvec[:], dins['cvec'][:])
        selcf = persist.tile([65, 3, NC_CH * 128], bf16)
        nc.sync.dma_start(selcf[:], dins['selcf'][:])
        onesbf = persist.tile([128, 1], bf16)
        nc.sync.dma_start(onesbf[:], dins['onesbf'][:])
        onesd = persist.tile([128, 1], bf16)
        nc.sync.dma_start(onesd[:], dins['onesd'][:])
        onesrow = persist.tile([1, PC], bf16)
        nc.sync.dma_start(onesrow[:], dins['onesrow'][:])
        epsc = persist.tile([1, 1], f32)
        nc.gpsimd.memset(epsc[:], EPS)
        clsw = persist.tile([128, NC_CH, C], bf16)
        nc.sync.dma_start(clsw[:], dins['clsw'][:])
        clsb = persist.tile([C, 1], f32)
        nc.sync.dma_start(clsb[:], dins['clsb'][:])
        clse = persist.tile([128, NC_CH], f32)
        nc.sync.dma_start(clse[:], dins['clse'][:])

        # -------------------------------------------------- front end
        with tc.tile_pool(name="fe", bufs=1) as fe, \
             tc.tile_pool(name="feps", bufs=2, space="PSUM") as feps:
            pint = fe.tile([E + 2, BPC * T * P], bf16)
            nc.sync.dma_start(pint[:], dins['pinT'][:])
            ballT = fe.tile([E + 2, BPC * T], bf16)
            nc.sync.dma_start(ballT[:], dins['ballT'][:])
            pw1 = fe.tile([E + 2, 128], bf16)
            nc.sync.dma_start(pw1[:], dins['pw1'][:])
            pw2 = fe.tile([128, NC_CH * 128], bf16)
            nc.sync.dma_start(pw2[:], dins['pw2'][:])
            bw1 = fe.tile([E + 2, 128], bf16)
            nc.sync.dma_start(bw1[:], dins['bw1'][:])
            bw2 = fe.tile([128, NC_CH * 128], bf16)
            nc.sync.dma_start(bw2[:], dins['bw2'][:])
            pb1 = fe.tile([128, 1], f32)
            nc.sync.dma_start(pb1[:], dins['pb1'][:])
            pb2s = fe.tile([128, NC_CH], f32)
            nc.sync.dma_start(pb2s[:], dins['pb2s'][:])
            bb1 = fe.tile([128, 1], f32)
            nc.sync.dma_start(bb1[:], dins['bb1'][:])
            bb2s = fe.tile([128, NC_CH], f32)
            nc.sync.dma_start(bb2s[:], dins['bb2s'][:])

            for pi in range(NPAIR):
                h1p = feps.tile([128, 400], f32, tag="feps")
                nc.tensor.matmul(h1p[:], pw1[:], pint[:, pi * 400:(pi + 1) * 400],
                                 start=True, stop=True)
                h1b = fe.tile([128, 400], bf16, tag="h1b")
                nc.vector.tensor_scalar(h1b[:], h1p[:], pb1[:, 0:1], 0.0,
                                        ALU.add, ALU.max)
                for c in range(NC_CH):
                    pfp = feps.tile([128, 400], f32, tag="feps")
                    nc.tensor.matmul(pfp[:], pw2[:, c * 128:(c + 1) * 128], h1b[:],
                                     start=True, stop=True)
                    Xc = X[:, c, :].rearrange("p (j k) -> p j k",
                                              j=BPC * T, k=P + 2)
                    dst = Xc[:, 2 * pi * T:(2 * pi + 2) * T, 0:P]
                    nc.scalar.activation(dst, pfp[:].rearrange(
                        "p (j k) -> p j k", j=2 * T, k=P),
                        AF.Identity, bias=pb2s[:, c:c + 1], scale=SCALE)
            h1bl = feps.tile([128, 320], f32, tag="feps")
            nc.tensor.matmul(h1bl[:], bw1[:], ballT[:], start=True, stop=True)
            h1blb = fe.tile([128, 320], bf16, tag="h1b")
            nc.vector.tensor_scalar(h1blb[:], h1bl[:], bb1[:, 0:1], 0.0,
                                    ALU.add, ALU.max)
            for c in range(NC_CH):
                bfp = feps.tile([128, 320], f32, tag="feps")
                nc.tensor.matmul(bfp[:], bw2[:, c * 128:(c + 1) * 128], h1blb[:],
                                 start=True, stop=True)
                Xc = X[:, c, :].rearrange("p (j k) -> p j k", j=BPC * T, k=P + 2)
                src = bfp[:].rearrange("p (j k) -> p j k", j=BPC * T, k=1)
                nc.scalar.activation(Xc[:, :, P:P + 1], src,
                                     AF.Identity, bias=bb2s[:, c:c + 1], scale=SCALE)
                nc.scalar.activation(Xc[:, :, P + 1:P + 2], src,
                                     AF.Identity, bias=clse[:, c:c + 1], scale=0.0)

        # -------------------------------------------------- pipelined layers
        wpool = ctx.enter_context(tc.tile_pool(name="wq", bufs=2))
        wpool1 = ctx.enter_context(tc.tile_pool(name="wf", bufs=1))
        spool = ctx.enter_context(tc.tile_pool(name="scr", bufs=1))
        rpool = ctx.enter_context(tc.tile_pool(name="ring", bufs=2))
        spool2 = ctx.enter_context(tc.tile_pool(name="scr2", bufs=2))
        tpool = ctx.enter_context(tc.tile_pool(name="tmp", bufs=4))
        ups = ctx.enter_context(tc.tile_pool(name="ups", bufs=4, space="PSUM"))
        ln_ps = ctx.enter_context(tc.tile_pool(name="lnps", bufs=2, space="PSUM"))
        sc_ps = ctx.enter_context(tc.tile_pool(name="scps", bufs=2, space="PSUM"))

        W = {}

        def emit_A1(l, pi):
            """qkv + v projections for stream A."""
            pc = slice(pi * PC, (pi + 1) * PC)
            dsc = qdescale[l]
            qkb = spool.tile([128, 8, PC], bf16, tag="qkb")
            for j in range(8):
                ps = ups.tile([128, 512], f32, tag="u")
                if l == 0:
                    for c in range(NC_CH):
                        nc.tensor.matmul(ps[:, :PC],
                                         W['wq0'][:, c, j * 128:(j + 1) * 128],
                                         X[:, c, pc],
                                         start=(c == 0), stop=(c == NC_CH - 1))
                    nc.scalar.activation(qkb[:, j, :], ps[:, :PC], AF.Identity,
                                         bias=cvec[:, l, j:j + 1])
                else:
                    n = 0
                    for t in range(2):
                        for kk in range(2):
                            nc.tensor.matmul(
                                ps[:, :PC],
                                W['wq8'][:, kk, t, :, j * 128:(j + 1) * 128],
                                XQ8[:, 2 * kk:2 * kk + 2, pc],
                                start=(n == 0), stop=(n == 3),
                                perf_mode=PM.DoubleRow)
                            n += 1
                    nc.scalar.activation(qkb[:, j, :], ps[:, :PC], AF.Identity,
                                         bias=cvec[:, l, j:j + 1], scale=dsc)
            vtm = spool.tile([128, 4, D], bf16, tag="vtm")
            for s4 in range(4):
                b, hf = divmod(s4, 2)
                ps = ups.tile([128, 512], f32, tag="u")
                if l == 0:
                    lo0 = pi * PC + b * S + hf * TK
                    for c in range(NC_CH):
                        nc.tensor.matmul(ps[:TK, :], X[:, c, lo0:lo0 + TK],
                                         W['wq0'][:, c, 2 * D:3 * D],
                                         start=(c == 0), stop=(c == NC_CH - 1))
                    nc.scalar.activation(vtm[:TK, s4, :], ps[:TK, :], AF.Identity)
                else:
                    glo = pi * PC + b * S + hf * TK
                    n = 0
                    for t in range(2):
                        for kk in range(2):
                            nc.tensor.matmul(
                                ps[:TK, :],
                                XQ8[:, 2 * kk:2 * kk + 2, glo:glo + TK],
                                W['wq8'][:, kk, t, :, 2 * D:3 * D],
                                start=(n == 0), stop=(n == 3),
                                perf_mode=PM.DoubleRow)
                            n += 1
                    nc.scalar.activation(vtm[:TK, s4, :], ps[:TK, :], AF.Identity,
                                         scale=dsc)
            return {'qkb': qkb, 'vtm': vtm, 'pc': pc, 'l': l, 'pi': pi}

        def emit_A2(st_):
            """scores -> exp -> mask(Pool) -> rowsums -> recip."""
            qkb = st_['qkb']
            eT = rpool.tile([TK, 32, S], bf16, tag="big16")
            st_['eT'] = eT
            for h in range(H):
                hb = (h % 2) * 64
                jq, jk = h // 2, 4 + h // 2
                for b in range(2):
                    sp = sc_ps.tile([TK, 512], f32, tag="sc")
                    for s in range(2):
                        nc.tensor.matmul(
                            sp[:, s * S:(s + 1) * S],
                            qkb[hb:hb + 64, jk,
                                b * S + s * TK:b * S + s * TK + TK],
                            qkb[hb:hb + 64, jq, b * S:(b + 1) * S],
                            start=True, stop=True)
                    tmp = tpool.tile([TK, 2, S], bf16, tag="exp")
                    nc.scalar.activation(
                        tmp[:],
                        sp[:, 0:2 * S].rearrange("p (s n) -> p s n", s=2),
                        AF.Exp)
                    nc.vector.tensor_tensor(
                        eT[:, 4 * h + 2 * b:4 * h + 2 * b + 2, :],
                        tmp[:], maskb[:], ALU.mult)
            rsts = []
            for i in range(3):
                rsts.append(ups.tile([65, 512], f32, tag="u", name=f"rs{i}"))
            eTh = eT[:].rearrange("p (h x) n -> p h x n", h=H)
            for h in range(H):
                ti, sub = HMAP[h]
                base = sub * 32
                for s in range(2):
                    nc.tensor.matmul(
                        rsts[ti][base:base + 1, :PC],
                        onesbf[:TK, 0:1],
                        eTh[:, h, s:4:2, :],
                        start=(s == 0), stop=(s == 1))
            stg = spool.tile([65, 3, PC], bf16, tag="stg")
            for ti in range(3):
                nc.gpsimd.memset(stg[:, ti, :], 1.0)
            with nc.allow_low_precision(reason="softmax recip"):
                for ti in range(3):
                    nc.vector.reciprocal(stg[:, ti, :], rsts[ti][:, :PC])
            st_['stg'] = stg

        def emit_A3(st_):
            """bc -> PV -> ofm ; Wo+bias ; residual STT ; LN1."""
            l, pc = st_['l'], st_['pc']
            eT, stg, vtm = st_['eT'], st_['stg'], st_['vtm']
            ofm = spool.tile([128, NC_CH, PC], bf16, tag="ofm")
            for c in range(NC_CH):
                bc = ups.tile([128, 512], f32, tag="u")
                tis = BC_TILES[c]
                for n, ti in enumerate(tis):
                    nc.tensor.matmul(bc[:, :PC],
                                     selcf[:, ti, c * 128:(c + 1) * 128],
                                     stg[:, ti, :],
                                     start=(n == 0), stop=(n == len(tis) - 1))
                bcs = spool.tile([128, PC], f32, tag="bcs")
                nc.scalar.activation(bcs[:], bc[:, :PC], AF.Identity)
                for b in range(2):
                    po = ups.tile([128, 512], f32, tag="u")
                    for hh in range(2):
                        h = 2 * c + hh
                        for s in range(2):
                            nc.tensor.matmul(
                                po[hh * 64:hh * 64 + 64, :S],
                                vtm[:TK, b * 2 + s, h * 64:(h + 1) * 64],
                                eT[:TK, 4 * h + 2 * b + s, :],
                                start=(s == 0), stop=(s == 1))
                    nc.vector.tensor_tensor(ofm[:, c, b * S:(b + 1) * S],
                                            bcs[:, b * S:(b + 1) * S],
                                            po[:, :S], ALU.mult)
            y = rpool.tile([128, NC_CH, PC], f32, tag="y")
            for c in range(NC_CH):
                ps = ups.tile([128, 512], f32, tag="u")
                nc.tensor.matmul(ps[:, :PC], W['brow'][0:1, c, :],
                                 onesrow[0:1, :], start=True, stop=False)
                for c2 in range(NC_CH):
                    nc.tensor.matmul(ps[:, :PC],
                                     W['wo'][:, c2, c * 128:(c + 1) * 128],
                                     ofm[:, c2, :],
                                     start=False, stop=(c2 == NC_CH - 1))
                nc.vector.scalar_tensor_tensor(y[:, c, :], X[:, c, pc],
                                               cvec[:, l, 8 + c:9 + c],
                                               ps[:, :PC], ALU.mult, ALU.add)
            _layernorm(nc, spool, rpool, ups, ln_ps, onesd, onesrow, epsc, y, X, pc)

        def emit_B1(l, pi):
            """FFN1 + relu (Pool)."""
            pc = slice(pi * PC, (pi + 1) * PC)
            hb_t = rpool.tile([128, NF_CH, PC], bf16, tag="big16")
            for fch in range(NF_CH):
                ps = ups.tile([128, 512], f32, tag="u")
                for c in range(NC_CH):
                    nc.tensor.matmul(ps[:, :PC],
                                     W['wf1'][:, c, fch * 128:(fch + 1) * 128],
                                     X[:, c, pc],
                                     start=(c == 0), stop=(c == NC_CH - 1))
                nc.scalar.activation(hb_t[:, fch, :], ps[:, :PC], AF.Relu,
                                      bias=cvec[:, l, 16 + fch:17 + fch])
            return {'hb': hb_t, 'pc': pc, 'l': l, 'pi': pi}

        def emit_B2(st_, last_layer):
            """FFN2 + bias ; residual STT ; LN2 ; fp8 shadow cast."""
            l, pc, hb_t = st_['l'], st_['pc'], st_['hb']
            y = rpool.tile([128, NC_CH, PC], f32, tag="y")
            for c in range(NC_CH):
                ps = ups.tile([128, 512], f32, tag="u")
                nc.tensor.matmul(ps[:, :PC], st_['brow'][0:1, NC_CH + c, :],
                                 onesrow[0:1, :], start=True, stop=False)
                for fch in range(NF_CH):
                    nc.tensor.matmul(ps[:, :PC],
                                     W['wf2'][:, fch, c * 128:(c + 1) * 128],
                                     hb_t[:, fch, :],
                                     start=False, stop=(fch == NF_CH - 1))
                nc.vector.scalar_tensor_tensor(y[:, c, :], X[:, c, pc],
                                               cvec[:, l, 12 + c:13 + c],
                                               ps[:, :PC], ALU.mult, ALU.add)
            _layernorm(nc, spool, rpool, ups, ln_ps, onesd, onesrow, epsc, y, X, pc)
            if not last_layer:
                nc.gpsimd.tensor_scalar(XQ8[:, :, pc], X[:, :, pc],
                                        SX, None, ALU.mult)

        nslots = n_layers * n_pairs
        stB = None
        for k in range(nslots + 1):
            newA = None
            if k < nslots:
                l, pi = divmod(k, n_pairs)
                if pi == 0:
                    if l == 0:
                        W['wq0'] = wpool1.tile([128, NC_CH, 3 * D], bf16,
                                               tag="wqf", name="wq0")
                        nc.sync.dma_start(W['wq0'][:], dins['wq0'][:])
                    else:
                        W['wq8'] = wpool1.tile([128, 2, 2, 2, 3 * D], fp8,
                                               tag="wqf", name="wq8")
                        nc.sync.dma_start(W['wq8'][:], dins['wq8'][l])
                    W['wo'] = wpool.tile([128, NC_CH, D], bf16, tag="wo",
                                         name="wo")
                    nc.sync.dma_start(W['wo'][:], dins['wo'][l])
                    W['brow'] = wpool.tile([1, 2 * NC_CH, 128], bf16,
                                           tag="brow", name="brow")
                    nc.sync.dma_start(W['brow'][:], dins['brows'][l])
                if pi == min(1, n_pairs - 1):
                    W['wf1'] = wpool1.tile([128, NC_CH, F], bf16, tag="wf1",
                                           name="wf1")
                    nc.sync.dma_start(W['wf1'][:], dins['wf1'][l])
                    W['wf2'] = wpool1.tile([128, NF_CH, D], bf16, tag="wf2",
                                           name="wf2")
                    nc.sync.dma_start(W['wf2'][:], dins['wf2'][l])
                newA = emit_A1(l, pi)
            if stB is not None:
                stB = dict(stB, **emit_B1(stB['l'], stB['pi']))
            if newA is not None:
                emit_A2(newA)
            if stB is not None:
                emit_B2(stB, stB['l'] == n_layers - 1)
                stB = None
            if newA is not None:
                emit_A3(newA)
                stB = {'l': newA['l'], 'pi': newA['pi'], 'brow': W['brow']}

        # -------------------------------------------------- classifier
        psc = ups.tile([C, 512], f32, tag="u")
        for c in range(NC_CH):
            nc.tensor.matmul(psc[:, :BPC], clsw[:, c, :],
                             X[:, c, :].rearrange("p (b t) -> p b t", b=BPC, t=S)
                             [:, :, S - 1],
                             start=(c == 0), stop=(c == NC_CH - 1))
        osb = spool.tile([C, BPC], f32, tag="osb")
        nc.scalar.activation(osb[:], psc[:, :BPC], AF.Identity, bias=clsb[:, 0:1])
        nc.sync.dma_start(dout[:], osb[:])


def _layernorm(nc, spool, rpool, ups, ln_ps, onesd, onesrow, epsc, y, X, pc):
    """X[:, :, pc] <- (y - mu) * rstd   (gain/bias folded into weights).
    onesd = 1/D column so the stats matmuls produce mu / m2 directly."""
    ybf = rpool.tile([128, NC_CH, PC], bf16, tag="ybf")
    nc.scalar.activation(ybf[:], y[:], AF.Identity)
    ysq = rpool.tile([128, NC_CH, PC], bf16, tag="ysq")
    nc.vector.tensor_tensor(ysq[:], ybf[:], ybf[:], ALU.mult)
    psA = ups.tile([1, 512], f32, tag="u", name="psA")
    for c in range(NC_CH):
        nc.tensor.matmul(psA[0:1, :PC], onesd[:, 0:1], ybf[:, c, :],
                         start=(c == 0), stop=(c == NC_CH - 1))
    psB = ups.tile([1, 512], f32, tag="u", name="psB")
    for c in range(NC_CH):
        nc.tensor.matmul(psB[0:1, :PC], onesd[:, 0:1], ysq[:, c, :],
                         start=(c == 0), stop=(c == NC_CH - 1))
    st = rpool.tile([1, 3, PC], f32, tag="st")     # 0=mu 1=var/sd 2=musq
    nc.scalar.square(st[0:1, 2, :], psA[0:1, :PC])
    nc.vector.scalar_tensor_tensor(st[0:1, 1, :], psB[0:1, :PC], 0.0,
                                   st[0:1, 2, :], ALU.add, ALU.subtract)
    nc.scalar.activation(st[0:1, 1, :], st[0:1, 1, :], AF.Sqrt,
                         bias=epsc[0:1, 0:1])
    stbf = rpool.tile([1, 2, PC], bf16, tag="stbf")
    with nc.allow_low_precision(reason="ln rstd/mu broadcast"):
        nc.vector.reciprocal(stbf[0:1, 1, :], st[0:1, 1, :])
    nc.scalar.activation(stbf[0:1, 0, :], psA[0:1, :PC], AF.Identity)
    bcA = ln_ps.tile([128, PC], f32, tag="bc", name="bcA")
    nc.tensor.matmul(bcA[:], onesrow[0:1, 0:128], stbf[0:1, 1, :],
                     start=True, stop=True)
    bcB = ln_ps.tile([128, PC], f32, tag="bc", name="bcB")
    nc.tensor.matmul(bcB[:], onesrow[0:1, 0:128], stbf[0:1, 0, :],
                     start=True, stop=True)
    for c in range(NC_CH):
        nc.vector.tensor_tensor(y[:, c, :], y[:, c, :], bcB[:], ALU.subtract)
        nc.vector.tensor_tensor(X[:, c, pc], y[:, c, :], bcA[:], ALU.mult)


# ---------------------------------------------------------------- host side
_CACHED = {}


def _prep_consts(inputs):
    bf = ml_dtypes.bfloat16
    f32n = np.float32
    mask = _mask_np()
    maskbit = (mask == 0.0).astype(f32n)
    maskT = maskbit.T
    maskb = maskT.reshape(2, TK, S).transpose(1, 0, 2).astype(bf)

    def chunk_pm(vec, nch=NC_CH):
        return np.ascontiguousarray(vec.reshape(nch, 128).T)

    cons = {}
    cons['maskb'] = np.ascontiguousarray(maskb)
    cons['pw1'] = inputs['pW1'].astype(bf)
    cons['pw2'] = np.ascontiguousarray(
        inputs['pW2'].reshape(128, NC_CH * 128)).astype(bf)
    cons['bw1'] = inputs['bW1'].astype(bf)
    cons['bw2'] = np.ascontiguousarray(
        inputs['bW2'].reshape(128, NC_CH * 128)).astype(bf)
    cons['pb1'] = inputs['pb1'].reshape(128, 1).astype(f32n)
    cons['pb2s'] = (chunk_pm(inputs['pb2']) * SCALE).astype(f32n)
    cons['bb1'] = inputs['bb1'].reshape(128, 1).astype(f32n)
    cons['bb2s'] = (chunk_pm(inputs['bb2']) * SCALE).astype(f32n)

    g1 = inputs['ln1g']; b1 = inputs['ln1b']
    g2 = inputs['ln2g']; b2 = inputs['ln2b']

    # effective weights with LN gains folded in
    wq_eff = np.empty_like(inputs['Wqkv'])
    bq_eff = np.empty_like(inputs['bqkv'])
    for l in range(L):
        gin = np.ones(D, f32n) if l == 0 else g2[l - 1]
        bin_ = np.zeros(D, f32n) if l == 0 else b2[l - 1]
        w = inputs['Wqkv'][l] * gin[:, None]
        bq = inputs['bqkv'][l] + bin_ @ inputs['Wqkv'][l]
        w[:, :D] *= HSCALE
        bq[:D] *= HSCALE
        wq_eff[l] = w
        bq_eff[l] = bq

    cons['wq0'] = np.ascontiguousarray(
        wq_eff[0].reshape(NC_CH, 128, 3 * D).transpose(1, 0, 2)).astype(bf)

    qdescale = [1.0] * L
    wq8 = np.zeros((L, 128, 2, 2, 2, 3 * D), e4np)
    for l in range(1, L):
        w = wq_eff[l]
        sw = 192.0 / max(np.abs(w).max(), 1e-9)
        qdescale[l] = 1.0 / (SX * sw)
        ws = (w * sw).astype(f32n)
        hi = ws.astype(e4np)
        lo = (ws - hi.astype(f32n)).astype(e4np)
        for kk in range(2):
            for i in range(2):
                ch = (2 * kk + i)
                wq8[l, :, kk, 0, i, :] = hi[ch * 128:(ch + 1) * 128, :]
                wq8[l, :, kk, 1, i, :] = lo[ch * 128:(ch + 1) * 128, :]
    cons['wq8'] = wq8

    def wlay(w, nch):
        Lw, K, N = w.shape
        return np.ascontiguousarray(
            w.reshape(Lw, nch, 128, N).transpose(0, 2, 1, 3)).astype(bf)

    wf1_eff = inputs['Wf1'] * g1[:, :, None]
    cons['wo'] = wlay(inputs['Wo'], NC_CH)
    cons['wf1'] = wlay(wf1_eff, NC_CH)
    cons['wf2'] = wlay(inputs['Wf2'], NF_CH)

    brows = np.zeros((L, 1, 2 * NC_CH, 128), f32n)
    cvec = np.zeros((128, L, 32), f32n)
    for l in range(L):
        bres = np.zeros(D, f32n) if l == 0 else b2[l - 1]
        gres = np.ones(D, f32n) if l == 0 else g2[l - 1]
        bo_eff = (inputs['bo'][l] + bq_eff[l][2 * D:] @ inputs['Wo'][l]
                  + bres)
        bf2_eff = inputs['bf2'][l] + b1[l]
        brows[l, 0, :NC_CH] = bo_eff.reshape(NC_CH, 128)
        brows[l, 0, NC_CH:] = bf2_eff.reshape(NC_CH, 128)
        cvec[:, l, 0:8] = np.ascontiguousarray(
            bq_eff[l][:2 * D].reshape(8, 128).T)
        cvec[:, l, 8:12] = chunk_pm(gres)
        cvec[:, l, 12:16] = chunk_pm(g1[l])
        cvec[:, l, 16:32] = np.ascontiguousarray(
            (inputs['bf1'][l] + b1[l] @ inputs['Wf1'][l]).reshape(NF_CH, 128).T)
    cons['brows'] = brows.astype(bf)
    cons['cvec'] = cvec

    selcf = np.zeros((65, 3, NC_CH * 128), f32n)
    for h in range(H):
        ti, sub = HMAP[h]
        c, half = divmod(h, 2)
        selcf[sub * 32, ti, c * 128 + half * 64: c * 128 + half * 64 + 64] = 1.0
    cons['selcf'] = selcf.astype(bf)
    cons['onesbf'] = np.ones((128, 1), bf)
    cons['onesd'] = np.full((128, 1), 1.0 / D, bf)
    cons['onesrow'] = np.ones((1, PC), bf)

    clsw_eff = inputs['clsW'] * g2[L - 1][:, None]
    clsb_eff = inputs['clsb'] + b2[L - 1] @ inputs['clsW']
    cons['clsw'] = np.ascontiguousarray(
        clsw_eff.reshape(NC_CH, 128, C).transpose(1, 0, 2)).astype(bf)
    cons['clsb'] = clsb_eff.reshape(C, 1).astype(f32n)
    cons['clse'] = chunk_pm(inputs['cls_e']).astype(f32n)
    return cons, qdescale


def kernel(**inputs):
    inputs = {k: np.asarray(v) for k, v in inputs.items()}
    bf = ml_dtypes.bfloat16
    cons, qdescale = _prep_consts(inputs)
    if 'nc' not in _CACHED:
        _CACHED['nc'] = build(tuple(qdescale))
    nc = _CACHED['nc']

    emb = inputs['emb'].astype(np.float32)
    pe = emb[inputs['player_idxs'].astype(np.int64)]
    pin = np.concatenate([pe,
                          inputs['player_xs'][..., None],
                          inputs['player_ys'][..., None]], -1)
    ball_e = np.broadcast_to(inputs['ball_e'], (B, T, E))
    bi = np.concatenate([ball_e,
                         inputs['ball_xs'][..., None],
                         inputs['ball_ys'][..., None]], -1)

    in_maps = []
    for core in range(NCORES):
        bs = slice(core * BPC, (core + 1) * BPC)
        m = dict(cons)
        m['pinT'] = np.ascontiguousarray(
            pin[bs].reshape(BPC * T * P, E + 2).T).astype(bf)
        m['ballT'] = np.ascontiguousarray(
            bi[bs].reshape(BPC * T, E + 2).T).astype(bf)
        in_maps.append(m)

    res = run_bass_kernel_spmd(nc, in_maps, core_ids=list(range(NCORES)))
    outs = [res.results[c]['out'] for c in range(NCORES)]
    full = np.concatenate([o.T for o in outs], axis=0)
    return full.astype(np.float32)


if __name__ == "__main__":
    nc = build(n_layers=1, n_pairs=1)
    print("build ok")


# revision 3
# speedup vs baseline: 1.0782x; 1.0782x over previous
"""Trainium2 Bass kernel for the Baller2Vec-style transformer encoder, v2.

Data parallel over batch (B=128) across 8 NeuronCores (16 per core, 8 pairs
of 2).  Feature-major activations.  The residual stream stores the
*unaffined* LayerNorm output (X-hat); each LN's per-feature gain/bias is
folded host-side into the adjacent weights and bias rows, removing three
DVE passes per LN.  QKV+V projections run as fp8(e4m3) DoubleRow matmuls
(weights hi+lo 2-term, activations hi) at 2x PE throughput; the rest is
bf16 with fp32 PSUM.  Sublayer biases enter PSUM via rank-1 matmuls;
softmax row-sum reciprocals are batched 3 heads per instruction; exp is
batched over 2-bank PSUM score tiles (4 segments per Activation op); FFN
relu runs on the otherwise idle Pool engine.
"""
import sys
import numpy as np

sys.path.insert(0, '/opt/trn_rl_repo')

import ml_dtypes
import concourse.bass as bass
import concourse.mybir as mybir
from concourse import tile
from concourse.bass_utils import run_bass_kernel_spmd
from concourse.vector_clock import ScopedClock

# ---------------------------------------------------------------- constants
HMAP = {0: (0, 0), 1: (0, 1), 2: (1, 0), 3: (1, 1), 4: (2, 0), 5: (2, 1),
        6: (0, 2), 7: (1, 2)}
BC_TILES = {0: [0], 1: [1], 2: [2], 3: [0, 1]}
P, T, E, D, H, F, L, V, C = 10, 20, 64, 512, 8, 2048, 6, 512, 9
B = 128
S = (P + 2) * T          # 240
HD = D // H
SCALE = float(np.sqrt(D))
HSCALE = 1.0 / float(np.sqrt(HD))
NCORES = 8
BPC = B // NCORES
NPAIR = BPC // 2
PC = 2 * S               # 480
NC_CH = D // 128
NF_CH = F // 128
TK = 120
EPS = 1e-5
SX = 16.0                # fp8 activation scale

bf16 = mybir.dt.bfloat16
f32 = mybir.dt.float32
fp8 = mybir.dt.float8e4
AF = mybir.ActivationFunctionType
ALU = mybir.AluOpType
PM = mybir.MatmulPerfMode
e4np = ml_dtypes.float8_e4m3fn


def _mask_np():
    m = np.full((S, S), -np.inf, dtype=np.float32)
    bs, cs = P * T, P * T + T
    for s in range(T):
        a, b_ = P * s, P * s + P
        m[a:b_, :b_] = 0.0
        m[a:b_, bs:bs + s + 1] = 0.0
        m[a:b_, cs:cs + s + 1] = 0.0
        for r in (bs + s, cs + s):
            m[r, :b_] = 0.0
            m[r, bs:bs + s + 1] = 0.0
            m[r, cs:cs + s + 1] = 0.0
    return m


# ---------------------------------------------------------------- tile patch
MAX_WAITS_PER_INST = 1


def _patched_drain_and_barrier(self, tick_clock, wait_clock):
    nc = self.nc
    probe = nc.sync.nop(nofuse=True)
    wait_clock.add_sem_waits(probe.ins, ScopedClock({None: tick_clock.global_clock}))
    si = probe.ins.sync_info
    waits = list(si.on_wait) if si is not None else []
    K = MAX_WAITS_PER_INST
    probe.ins.sync_info = mybir.SyncInfo(on_wait=waits[:K], on_update=[])
    for j in range(K, len(waits), K):
        w = nc.sync.nop(nofuse=True)
        w.ins.sync_info = mybir.SyncInfo(on_wait=waits[j:j + K], on_update=[])
    nc.sync.drain()
    nc.all_engine_barrier()
    popped = nc._tile_sem_poison_stack.pop()
    assert popped is self._sem_poison
    nc.clear_and_free_semaphores(list(self.sems.allocated().values()))
    nc.all_engine_barrier()


tile.TileContext._drain_and_barrier = _patched_drain_and_barrier


def _split_waits(nc, K=1):
    wsid = [0]
    for bb in nc.m.functions[0].blocks:
        out = []
        changed = False
        for inst in bb.instructions:
            si = inst.sync_info
            waits = list(si.on_wait) if si is not None else []
            if len(waits) > K:
                changed = True
                extra = waits[:-K]
                for j in range(0, len(extra), K):
                    nop = mybir.InstNoOp(name=f"WSNOP-{wsid[0]}")
                    wsid[0] += 1
                    nop.engine = inst.engine
                    nop.sync_info = mybir.SyncInfo(on_wait=extra[j:j + K],
                                                   on_update=[])
                    out.append(nop)
                inst.sync_info = mybir.SyncInfo(on_wait=waits[-K:],
                                                on_update=list(si.on_update))
            out.append(inst)
        if changed:
            bb.instructions = out


# ---------------------------------------------------------------- builder
def build(qdescale=(1.0,) * L, n_layers=L, n_pairs=NPAIR):
    """qdescale[l]: immediate multiplier applied when copying the fp8
    QKV/V psum back to bf16 (1/(sx*sw)); qdescale[0] unused."""
    nc = bass.Bass("TRN2", target_bir_lowering=False)
    dt_in = {
        'pinT': ([E + 2, BPC * T * P], bf16),
        'ballT': ([E + 2, BPC * T], bf16),
        'maskb': ([TK, 2, S], bf16),
        'pw1': ([E + 2, 128], bf16),
        'pw2': ([128, NC_CH * 128], bf16),
        'bw1': ([E + 2, 128], bf16),
        'bw2': ([128, NC_CH * 128], bf16),
        'pb1': ([128, 1], f32),
        'pb2s': ([128, NC_CH], f32),
        'bb1': ([128, 1], f32),
        'bb2s': ([128, NC_CH], f32),
        'wq0': ([128, NC_CH, 3 * D], bf16),
        'wq8': ([L, 128, 2, 2, 2, 3 * D], fp8),
        'wo': ([L, 128, NC_CH, D], bf16),
        'wf1': ([L, 128, NC_CH, F], bf16),
        'wf2': ([L, 128, NF_CH, D], bf16),
        'brows': ([L, 1, 2 * NC_CH, 128], bf16),
        'cvec': ([128, L, 32], f32),
        'selcf': ([65, 3, NC_CH * 128], bf16),
        'onesbf': ([128, 1], bf16),
        'onesd': ([128, 1], bf16),
        'onesrow': ([1, PC], bf16),
        'clsw': ([128, NC_CH, C], bf16),
        'clsb': ([C, 1], f32),
        'clse': ([128, NC_CH], f32),
    }
    dins = {k: nc.dram_tensor(k, sh, dt, kind="ExternalInput")
            for k, (sh, dt) in dt_in.items()}
    dout = nc.dram_tensor("out", [C, BPC], f32, kind="ExternalOutput")

    with tile.TileContext(nc) as tc:
        _body(nc, tc, dins, dout, qdescale, n_layers, n_pairs)
    _split_waits(nc)
    return nc


def _body(nc, tc, dins, dout, qdescale, n_layers, n_pairs):
    import contextlib
    ctx = contextlib.ExitStack()
    with ctx:
        persist = ctx.enter_context(tc.tile_pool(name="persist", bufs=1))
        X = persist.tile([128, NC_CH, BPC * S], bf16)     # residual (X-hat)
        XQ8 = persist.tile([128, NC_CH, BPC * S], fp8)    # fp8 shadow for qkv

        maskb = persist.tile([TK, 2, S], bf16)
        nc.sync.dma_start(maskb[:], dins['maskb'][:])
        cvec = persist.tile([128, L, 32], f32)
        nc.sync.dma_start(cvec[:], dins['cvec'][:])
        selcf = persist.tile([65, 3, NC_CH * 128], bf16)
        nc.sync.dma_start(selcf[:], dins['selcf'][:])
        onesbf = persist.tile([128, 1], bf16)
        nc.sync.dma_start(onesbf[:], dins['onesbf'][:])
        onesd = persist.tile([128, 1], bf16)
        nc.sync.dma_start(onesd[:], dins['onesd'][:])
        onesrow = persist.tile([1, PC], bf16)
        nc.sync.dma_start(onesrow[:], dins['onesrow'][:])
        epsc = persist.tile([1, 1], f32)
        nc.gpsimd.memset(epsc[:], EPS)
        clsw = persist.tile([128, NC_CH, C], bf16)
        nc.sync.dma_start(clsw[:], dins['clsw'][:])
        clsb = persist.tile([C, 1], f32)
        nc.sync.dma_start(clsb[:], dins['clsb'][:])
        clse = persist.tile([128, NC_CH], f32)
        nc.sync.dma_start(clse[:], dins['clse'][:])

        # -------------------------------------------------- front end
        with tc.tile_pool(name="fe", bufs=1) as fe, \
             tc.tile_pool(name="feps", bufs=2, space="PSUM") as feps:
            pint = fe.tile([E + 2, BPC * T * P], bf16)
            nc.sync.dma_start(pint[:], dins['pinT'][:])
            ballT = fe.tile([E + 2, BPC * T], bf16)
            nc.sync.dma_start(ballT[:], dins['ballT'][:])
            pw1 = fe.tile([E + 2, 128], bf16)
            nc.sync.dma_start(pw1[:], dins['pw1'][:])
            pw2 = fe.tile([128, NC_CH * 128], bf16)
            nc.sync.dma_start(pw2[:], dins['pw2'][:])
            bw1 = fe.tile([E + 2, 128], bf16)
            nc.sync.dma_start(bw1[:], dins['bw1'][:])
            bw2 = fe.tile([128, NC_CH * 128], bf16)
            nc.sync.dma_start(bw2[:], dins['bw2'][:])
            pb1 = fe.tile([128, 1], f32)
            nc.sync.dma_start(pb1[:], dins['pb1'][:])
            pb2s = fe.tile([128, NC_CH], f32)
            nc.sync.dma_start(pb2s[:], dins['pb2s'][:])
            bb1 = fe.tile([128, 1], f32)
            nc.sync.dma_start(bb1[:], dins['bb1'][:])
            bb2s = fe.tile([128, NC_CH], f32)
            nc.sync.dma_start(bb2s[:], dins['bb2s'][:])

            for pi in range(NPAIR):
                h1p = feps.tile([128, 400], f32, tag="feps")
                nc.tensor.matmul(h1p[:], pw1[:], pint[:, pi * 400:(pi + 1) * 400],
                                 start=True, stop=True)
                h1b = fe.tile([128, 400], bf16, tag="h1b")
                nc.vector.tensor_scalar(h1b[:], h1p[:], pb1[:, 0:1], 0.0,
                                        ALU.add, ALU.max)
                for c in range(NC_CH):
                    pfp = feps.tile([128, 400], f32, tag="feps")
                    nc.tensor.matmul(pfp[:], pw2[:, c * 128:(c + 1) * 128], h1b[:],
                                     start=True, stop=True)
                    Xc = X[:, c, :].rearrange("p (j k) -> p j k",
                                              j=BPC * T, k=P + 2)
                    dst = Xc[:, 2 * pi * T:(2 * pi + 2) * T, 0:P]
                    nc.scalar.activation(dst, pfp[:].rearrange(
                        "p (j k) -> p j k", j=2 * T, k=P),
                        AF.Identity, bias=pb2s[:, c:c + 1], scale=SCALE)
            h1bl = feps.tile([128, 320], f32, tag="feps")
            nc.tensor.matmul(h1bl[:], bw1[:], ballT[:], start=True, stop=True)
            h1blb = fe.tile([128, 320], bf16, tag="h1b")
            nc.vector.tensor_scalar(h1blb[:], h1bl[:], bb1[:, 0:1], 0.0,
                                    ALU.add, ALU.max)
            for c in range(NC_CH):
                bfp = feps.tile([128, 320], f32, tag="feps")
                nc.tensor.matmul(bfp[:], bw2[:, c * 128:(c + 1) * 128], h1blb[:],
                                 start=True, stop=True)
                Xc = X[:, c, :].rearrange("p (j k) -> p j k", j=BPC * T, k=P + 2)
                src = bfp[:].rearrange("p (j k) -> p j k", j=BPC * T, k=1)
                nc.scalar.activation(Xc[:, :, P:P + 1], src,
                                     AF.Identity, bias=bb2s[:, c:c + 1], scale=SCALE)
                nc.scalar.activation(Xc[:, :, P + 1:P + 2], src,
                                     AF.Identity, bias=clse[:, c:c + 1], scale=0.0)

        # -------------------------------------------------- pipelined layers
        wpool = ctx.enter_context(tc.tile_pool(name="wq", bufs=2))
        wpool1 = ctx.enter_context(tc.tile_pool(name="wf", bufs=1))
        spool = ctx.enter_context(tc.tile_pool(name="scr", bufs=1))
        rpool = ctx.enter_context(tc.tile_pool(name="ring", bufs=2))
        spool2 = ctx.enter_context(tc.tile_pool(name="scr2", bufs=2))
        tpool = ctx.enter_context(tc.tile_pool(name="tmp", bufs=4))
        ups = ctx.enter_context(tc.tile_pool(name="ups", bufs=4, space="PSUM"))
        ln_ps = ctx.enter_context(tc.tile_pool(name="lnps", bufs=2, space="PSUM"))
        sc_ps = ctx.enter_context(tc.tile_pool(name="scps", bufs=2, space="PSUM"))

        W = {}

        def emit_A1(l, pi):
            """qkv + v projections for stream A."""
            pc = slice(pi * PC, (pi + 1) * PC)
            dsc = qdescale[l]
            qkb = spool.tile([128, 8, PC], bf16, tag="qkb")
            for j in range(8):
                ps = ups.tile([128, 512], f32, tag="u")
                if l == 0:
                    for c in range(NC_CH):
                        nc.tensor.matmul(ps[:, :PC],
                                         W['wq0'][:, c, j * 128:(j + 1) * 128],
                                         X[:, c, pc],
                                         start=(c == 0), stop=(c == NC_CH - 1))
                    nc.scalar.activation(qkb[:, j, :], ps[:, :PC], AF.Identity,
                                         bias=cvec[:, l, j:j + 1])
                else:
                    n = 0
                    for t in range(2):
                        for kk in range(2):
                            nc.tensor.matmul(
                                ps[:, :PC],
                                W['wq8'][:, kk, t, :, j * 128:(j + 1) * 128],
                                XQ8[:, 2 * kk:2 * kk + 2, pc],
                                start=(n == 0), stop=(n == 3),
                                perf_mode=PM.DoubleRow)
                            n += 1
                    nc.scalar.activation(qkb[:, j, :], ps[:, :PC], AF.Identity,
                                         bias=cvec[:, l, j:j + 1], scale=dsc)
            vtm = spool.tile([128, 4, D], bf16, tag="vtm")
            for s4 in range(4):
                b, hf = divmod(s4, 2)
                ps = ups.tile([128, 512], f32, tag="u")
                if l == 0:
                    lo0 = pi * PC + b * S + hf * TK
                    for c in range(NC_CH):
                        nc.tensor.matmul(ps[:TK, :], X[:, c, lo0:lo0 + TK],
                                         W['wq0'][:, c, 2 * D:3 * D],
                                         start=(c == 0), stop=(c == NC_CH - 1))
                    nc.scalar.activation(vtm[:TK, s4, :], ps[:TK, :], AF.Identity)
                else:
                    glo = pi * PC + b * S + hf * TK
                    n = 0
                    for t in range(2):
                        for kk in range(2):
                            nc.tensor.matmul(
                                ps[:TK, :],
                                XQ8[:, 2 * kk:2 * kk + 2, glo:glo + TK],
                                W['wq8'][:, kk, t, :, 2 * D:3 * D],
                                start=(n == 0), stop=(n == 3),
                                perf_mode=PM.DoubleRow)
                            n += 1
                    nc.scalar.activation(vtm[:TK, s4, :], ps[:TK, :], AF.Identity,
                                         scale=dsc)
            return {'qkb': qkb, 'vtm': vtm, 'pc': pc, 'l': l, 'pi': pi}

        def emit_A2(st_):
            """scores -> exp -> mask(Pool) -> rowsums -> recip."""
            qkb = st_['qkb']
            eT = rpool.tile([TK, 32, S], bf16, tag="big16")
            st_['eT'] = eT
            for h in range(H):
                hb = (h % 2) * 64
                jq, jk = h // 2, 4 + h // 2
                for b in range(2):
                    sp = sc_ps.tile([TK, 512], f32, tag="sc")
                    for s in range(2):
                        nc.tensor.matmul(
                            sp[:, s * S:(s + 1) * S],
                            qkb[hb:hb + 64, jk,
                                b * S + s * TK:b * S + s * TK + TK],
                            qkb[hb:hb + 64, jq, b * S:(b + 1) * S],
                            start=True, stop=True)
                    tmp = tpool.tile([TK, 2, S], bf16, tag="exp")
                    nc.scalar.activation(
                        tmp[:],
                        sp[:, 0:2 * S].rearrange("p (s n) -> p s n", s=2),
                        AF.Exp)
                    nc.vector.tensor_tensor(
                        eT[:, 4 * h + 2 * b:4 * h + 2 * b + 2, :],
                        tmp[:], maskb[:], ALU.mult)
            rsts = []
            for i in range(3):
                rsts.append(ups.tile([65, 512], f32, tag="u", name=f"rs{i}"))
            eTh = eT[:].rearrange("p (h x) n -> p h x n", h=H)
            for h in range(H):
                ti, sub = HMAP[h]
                base = sub * 32
                for s in range(2):
                    nc.tensor.matmul(
                        rsts[ti][base:base + 1, :PC],
                        onesbf[:TK, 0:1],
                        eTh[:, h, s:4:2, :],
                        start=(s == 0), stop=(s == 1))
            stg = spool.tile([65, 3, PC], bf16, tag="stg")
            for ti in range(3):
                nc.gpsimd.memset(stg[:, ti, :], 1.0)
            with nc.allow_low_precision(reason="softmax recip"):
                for ti in range(3):
                    nc.vector.reciprocal(stg[:, ti, :], rsts[ti][:, :PC])
            st_['stg'] = stg

        def emit_A3(st_):
            """bc -> PV -> ofm ; Wo+bias ; residual STT ; LN1."""
            l, pc = st_['l'], st_['pc']
            eT, stg, vtm = st_['eT'], st_['stg'], st_['vtm']
            ofm = spool.tile([128, NC_CH, PC], bf16, tag="ofm")
            for c in range(NC_CH):
                bc = ups.tile([128, 512], f32, tag="u")
                tis = BC_TILES[c]
                for n, ti in enumerate(tis):
                    nc.tensor.matmul(bc[:, :PC],
                                     selcf[:, ti, c * 128:(c + 1) * 128],
                                     stg[:, ti, :],
                                     start=(n == 0), stop=(n == len(tis) - 1))
                bcs = spool.tile([128, PC], f32, tag="bcs")
                nc.scalar.activation(bcs[:], bc[:, :PC], AF.Identity)
                for b in range(2):
                    po = ups.tile([128, 512], f32, tag="u")
                    for hh in range(2):
                        h = 2 * c + hh
                        for s in range(2):
                            nc.tensor.matmul(
                                po[hh * 64:hh * 64 + 64, :S],
                                vtm[:TK, b * 2 + s, h * 64:(h + 1) * 64],
                                eT[:TK, 4 * h + 2 * b + s, :],
                                start=(s == 0), stop=(s == 1))
                    nc.vector.tensor_tensor(ofm[:, c, b * S:(b + 1) * S],
                                            bcs[:, b * S:(b + 1) * S],
                                            po[:, :S], ALU.mult)
            y = rpool.tile([128, NC_CH, PC], f32, tag="y")
            for c in range(NC_CH):
                ps = ups.tile([128, 512], f32, tag="u")
                nc.tensor.matmul(ps[:, :PC], W['brow'][0:1, c, :],
                                 onesrow[0:1, :], start=True, stop=False)
                for c2 in range(NC_CH):
                    nc.tensor.matmul(ps[:, :PC],
                                     W['wo'][:, c2, c * 128:(c + 1) * 128],
                                     ofm[:, c2, :],
                                     start=False, stop=(c2 == NC_CH - 1))
                nc.vector.scalar_tensor_tensor(y[:, c, :], X[:, c, pc],
                                               cvec[:, l, 8 + c:9 + c],
                                               ps[:, :PC], ALU.mult, ALU.add)
            _layernorm(nc, spool, rpool, ups, ln_ps, onesd, onesrow, epsc, y, X, pc)

        def emit_B1(l, pi):
            """FFN1 + relu (Pool)."""
            pc = slice(pi * PC, (pi + 1) * PC)
            hb_t = rpool.tile([128, NF_CH, PC], bf16, tag="big16")
            for fch in range(NF_CH):
                ps = ups.tile([128, 512], f32, tag="u")
                for c in range(NC_CH):
                    nc.tensor.matmul(ps[:, :PC],
                                     W['wf1'][:, c, fch * 128:(fch + 1) * 128],
                                     X[:, c, pc],
                                     start=(c == 0), stop=(c == NC_CH - 1))
                nc.scalar.activation(hb_t[:, fch, :], ps[:, :PC], AF.Relu,
                                      bias=cvec[:, l, 16 + fch:17 + fch])
            return {'hb': hb_t, 'pc': pc, 'l': l, 'pi': pi}

        def emit_B2a(st_):
            """FFN2 + bias ; residual STT."""
            l, pc, hb_t = st_['l'], st_['pc'], st_['hb']
            y = rpool.tile([128, NC_CH, PC], f32, tag="y")
            st_['y'] = y
            for c in range(NC_CH):
                ps = ups.tile([128, 512], f32, tag="u")
                nc.tensor.matmul(ps[:, :PC], st_['brow'][0:1, NC_CH + c, :],
                                 onesrow[0:1, :], start=True, stop=False)
                for fch in range(NF_CH):
                    nc.tensor.matmul(ps[:, :PC],
                                     W['wf2'][:, fch, c * 128:(c + 1) * 128],
                                     hb_t[:, fch, :],
                                     start=False, stop=(fch == NF_CH - 1))
                nc.vector.scalar_tensor_tensor(y[:, c, :], X[:, c, pc],
                                               cvec[:, l, 12 + c:13 + c],
                                               ps[:, :PC], ALU.mult, ALU.add)

        def emit_B2b(st_, last_layer):
            """LN2 ; fp8 shadow cast."""
            pc = st_['pc']
            _layernorm(nc, spool, rpool, ups, ln_ps, onesd, onesrow, epsc,
                       st_['y'], X, pc)
            if not last_layer:
                nc.gpsimd.tensor_scalar(XQ8[:, :, pc], X[:, :, pc],
                                        SX, None, ALU.mult)

        nslots = n_layers * n_pairs
        stB = None
        for k in range(nslots + 1):
            newA = None
            if k < nslots:
                l, pi = divmod(k, n_pairs)
                if pi == 0:
                    if l == 0:
                        W['wq0'] = wpool1.tile([128, NC_CH, 3 * D], bf16,
                                               tag="wqf", name="wq0")
                        nc.sync.dma_start(W['wq0'][:], dins['wq0'][:])
                    else:
                        W['wq8'] = wpool1.tile([128, 2, 2, 2, 3 * D], fp8,
                                               tag="wqf", name="wq8")
                        nc.sync.dma_start(W['wq8'][:], dins['wq8'][l])
                    W['wo'] = wpool.tile([128, NC_CH, D], bf16, tag="wo",
                                         name="wo")
                    nc.sync.dma_start(W['wo'][:], dins['wo'][l])
                    W['brow'] = wpool.tile([1, 2 * NC_CH, 128], bf16,
                                           tag="brow", name="brow")
                    nc.sync.dma_start(W['brow'][:], dins['brows'][l])
                if pi == min(1, n_pairs - 1):
                    W['wf1'] = wpool1.tile([128, NC_CH, F], bf16, tag="wf1",
                                           name="wf1")
                    nc.sync.dma_start(W['wf1'][:], dins['wf1'][l])
                    W['wf2'] = wpool1.tile([128, NF_CH, D], bf16, tag="wf2",
                                           name="wf2")
                    nc.sync.dma_start(W['wf2'][:], dins['wf2'][l])
                newA = emit_A1(l, pi)
            if stB is not None:
                stB = dict(stB, **emit_B1(stB['l'], stB['pi']))
            if newA is not None:
                emit_A2(newA)
            if stB is not None:
                emit_B2a(stB)
            if newA is not None:
                emit_A3(newA)
            if stB is not None:
                emit_B2b(stB, stB['l'] == n_layers - 1)
                stB = None
            if newA is not None:
                stB = {'l': newA['l'], 'pi': newA['pi'], 'brow': W['brow']}

        # -------------------------------------------------- classifier
        psc = ups.tile([C, 512], f32, tag="u")
        for c in range(NC_CH):
            nc.tensor.matmul(psc[:, :BPC], clsw[:, c, :],
                             X[:, c, :].rearrange("p (b t) -> p b t", b=BPC, t=S)
                             [:, :, S - 1],
                             start=(c == 0), stop=(c == NC_CH - 1))
        osb = spool.tile([C, BPC], f32, tag="osb")
        nc.scalar.activation(osb[:], psc[:, :BPC], AF.Identity, bias=clsb[:, 0:1])
        nc.sync.dma_start(dout[:], osb[:])


def _layernorm(nc, spool, rpool, ups, ln_ps, onesd, onesrow, epsc, y, X, pc):
    """X[:, :, pc] <- (y - mu) * rstd   (gain/bias folded into weights).
    onesd = 1/D column so the stats matmuls produce mu / m2 directly."""
    ybf = rpool.tile([128, NC_CH, PC], bf16, tag="ybf")
    nc.scalar.activation(ybf[:], y[:], AF.Identity)
    ysq = rpool.tile([128, NC_CH, PC], bf16, tag="ysq")
    nc.vector.tensor_tensor(ysq[:], ybf[:], ybf[:], ALU.mult)
    psA = ups.tile([1, 512], f32, tag="u", name="psA")
    for c in range(NC_CH):
        nc.tensor.matmul(psA[0:1, :PC], onesd[:, 0:1], ybf[:, c, :],
                         start=(c == 0), stop=(c == NC_CH - 1))
    psB = ups.tile([1, 512], f32, tag="u", name="psB")
    for c in range(NC_CH):
        nc.tensor.matmul(psB[0:1, :PC], onesd[:, 0:1], ysq[:, c, :],
                         start=(c == 0), stop=(c == NC_CH - 1))
    st = rpool.tile([1, 3, PC], f32, tag="st")     # 0=mu 1=var/sd 2=musq
    nc.scalar.square(st[0:1, 2, :], psA[0:1, :PC])
    nc.vector.scalar_tensor_tensor(st[0:1, 1, :], psB[0:1, :PC], 0.0,
                                   st[0:1, 2, :], ALU.add, ALU.subtract)
    nc.scalar.activation(st[0:1, 1, :], st[0:1, 1, :], AF.Sqrt,
                         bias=epsc[0:1, 0:1])
    stbf = rpool.tile([1, 2, PC], bf16, tag="stbf")
    with nc.allow_low_precision(reason="ln rstd/mu broadcast"):
        nc.vector.reciprocal(stbf[0:1, 1, :], st[0:1, 1, :])
    nc.scalar.activation(stbf[0:1, 0, :], psA[0:1, :PC], AF.Identity)
    bcA = ln_ps.tile([128, PC], f32, tag="bc", name="bcA")
    nc.tensor.matmul(bcA[:], onesrow[0:1, 0:128], stbf[0:1, 1, :],
                     start=True, stop=True)
    bcB = ln_ps.tile([128, PC], f32, tag="bc", name="bcB")
    nc.tensor.matmul(bcB[:], onesrow[0:1, 0:128], stbf[0:1, 0, :],
                     start=True, stop=True)
    for c in range(NC_CH):
        nc.vector.tensor_tensor(y[:, c, :], y[:, c, :], bcB[:], ALU.subtract)
        nc.vector.tensor_tensor(X[:, c, pc], y[:, c, :], bcA[:], ALU.mult)


# ---------------------------------------------------------------- host side
_CACHED = {}


def _prep_consts(inputs):
    bf = ml_dtypes.bfloat16
    f32n = np.float32
    mask = _mask_np()
    maskbit = (mask == 0.0).astype(f32n)
    maskT = maskbit.T
    maskb = maskT.reshape(2, TK, S).transpose(1, 0, 2).astype(bf)

    def chunk_pm(vec, nch=NC_CH):
        return np.ascontiguousarray(vec.reshape(nch, 128).T)

    cons = {}
    cons['maskb'] = np.ascontiguousarray(maskb)
    cons['pw1'] = inputs['pW1'].astype(bf)
    cons['pw2'] = np.ascontiguousarray(
        inputs['pW2'].reshape(128, NC_CH * 128)).astype(bf)
    cons['bw1'] = inputs['bW1'].astype(bf)
    cons['bw2'] = np.ascontiguousarray(
        inputs['bW2'].reshape(128, NC_CH * 128)).astype(bf)
    cons['pb1'] = inputs['pb1'].reshape(128, 1).astype(f32n)
    cons['pb2s'] = (chunk_pm(inputs['pb2']) * SCALE).astype(f32n)
    cons['bb1'] = inputs['bb1'].reshape(128, 1).astype(f32n)
    cons['bb2s'] = (chunk_pm(inputs['bb2']) * SCALE).astype(f32n)

    g1 = inputs['ln1g']; b1 = inputs['ln1b']
    g2 = inputs['ln2g']; b2 = inputs['ln2b']

    # effective weights with LN gains folded in
    wq_eff = np.empty_like(inputs['Wqkv'])
    bq_eff = np.empty_like(inputs['bqkv'])
    for l in range(L):
        gin = np.ones(D, f32n) if l == 0 else g2[l - 1]
        bin_ = np.zeros(D, f32n) if l == 0 else b2[l - 1]
        w = inputs['Wqkv'][l] * gin[:, None]
        bq = inputs['bqkv'][l] + bin_ @ inputs['Wqkv'][l]
        w[:, :D] *= HSCALE
        bq[:D] *= HSCALE
        wq_eff[l] = w
        bq_eff[l] = bq

    cons['wq0'] = np.ascontiguousarray(
        wq_eff[0].reshape(NC_CH, 128, 3 * D).transpose(1, 0, 2)).astype(bf)

    qdescale = [1.0] * L
    wq8 = np.zeros((L, 128, 2, 2, 2, 3 * D), e4np)
    for l in range(1, L):
        w = wq_eff[l]
        sw = 192.0 / max(np.abs(w).max(), 1e-9)
        qdescale[l] = 1.0 / (SX * sw)
        ws = (w * sw).astype(f32n)
        hi = ws.astype(e4np)
        lo = (ws - hi.astype(f32n)).astype(e4np)
        for kk in range(2):
            for i in range(2):
                ch = (2 * kk + i)
                wq8[l, :, kk, 0, i, :] = hi[ch * 128:(ch + 1) * 128, :]
                wq8[l, :, kk, 1, i, :] = lo[ch * 128:(ch + 1) * 128, :]
    cons['wq8'] = wq8

    def wlay(w, nch):
        Lw, K, N = w.shape
        return np.ascontiguousarray(
            w.reshape(Lw, nch, 128, N).transpose(0, 2, 1, 3)).astype(bf)

    wf1_eff = inputs['Wf1'] * g1[:, :, None]
    cons['wo'] = wlay(inputs['Wo'], NC_CH)
    cons['wf1'] = wlay(wf1_eff, NC_CH)
    cons['wf2'] = wlay(inputs['Wf2'], NF_CH)

    brows = np.zeros((L, 1, 2 * NC_CH, 128), f32n)
    cvec = np.zeros((128, L, 32), f32n)
    for l in range(L):
        bres = np.zeros(D, f32n) if l == 0 else b2[l - 1]
        gres = np.ones(D, f32n) if l == 0 else g2[l - 1]
        bo_eff = (inputs['bo'][l] + bq_eff[l][2 * D:] @ inputs['Wo'][l]
                  + bres)
        bf2_eff = inputs['bf2'][l] + b1[l]
        brows[l, 0, :NC_CH] = bo_eff.reshape(NC_CH, 128)
        brows[l, 0, NC_CH:] = bf2_eff.reshape(NC_CH, 128)
        cvec[:, l, 0:8] = np.ascontiguousarray(
            bq_eff[l][:2 * D].reshape(8, 128).T)
        cvec[:, l, 8:12] = chunk_pm(gres)
        cvec[:, l, 12:16] = chunk_pm(g1[l])
        cvec[:, l, 16:32] = np.ascontiguousarray(
            (inputs['bf1'][l] + b1[l] @ inputs['Wf1'][l]).reshape(NF_CH, 128).T)
    cons['brows'] = brows.astype(bf)
    cons['cvec'] = cvec

    selcf = np.zeros((65, 3, NC_CH * 128), f32n)
    for h in range(H):
        ti, sub = HMAP[h]
        c, half = divmod(h, 2)
        selcf[sub * 32, ti, c * 128 + half * 64: c * 128 + half * 64 + 64] = 1.0
    cons['selcf'] = selcf.astype(bf)
    cons['onesbf'] = np.ones((128, 1), bf)
    cons['onesd'] = np.full((128, 1), 1.0 / D, bf)
    cons['onesrow'] = np.ones((1, PC), bf)

    clsw_eff = inputs['clsW'] * g2[L - 1][:, None]
    clsb_eff = inputs['clsb'] + b2[L - 1] @ inputs['clsW']
    cons['clsw'] = np.ascontiguousarray(
        clsw_eff.reshape(NC_CH, 128, C).transpose(1, 0, 2)).astype(bf)
    cons['clsb'] = clsb_eff.reshape(C, 1).astype(f32n)
    cons['clse'] = chunk_pm(inputs['cls_e']).astype(f32n)
    return cons, qdescale


def kernel(**inputs):
    inputs = {k: np.asarray(v) for k, v in inputs.items()}
    bf = ml_dtypes.bfloat16
    cons, qdescale = _prep_consts(inputs)
    if 'nc' not in _CACHED:
        _CACHED['nc'] = build(tuple(qdescale))
    nc = _CACHED['nc']

    emb = inputs['emb'].astype(np.float32)
    pe = emb[inputs['player_idxs'].astype(np.int64)]
    pin = np.concatenate([pe,
                          inputs['player_xs'][..., None],
                          inputs['player_ys'][..., None]], -1)
    ball_e = np.broadcast_to(inputs['ball_e'], (B, T, E))
    bi = np.concatenate([ball_e,
                         inputs['ball_xs'][..., None],
                         inputs['ball_ys'][..., None]], -1)

    in_maps = []
    for core in range(NCORES):
        bs = slice(core * BPC, (core + 1) * BPC)
        m = dict(cons)
        m['pinT'] = np.ascontiguousarray(
            pin[bs].reshape(BPC * T * P, E + 2).T).astype(bf)
        m['ballT'] = np.ascontiguousarray(
            bi[bs].reshape(BPC * T, E + 2).T).astype(bf)
        in_maps.append(m)

    res = run_bass_kernel_spmd(nc, in_maps, core_ids=list(range(NCORES)))
    outs = [res.results[c]['out'] for c in range(NCORES)]
    full = np.concatenate([o.T for o in outs], axis=0)
    return full.astype(np.float32)


if __name__ == "__main__":
    nc = build(n_layers=1, n_pairs=1)
    print("build ok")


# revision 7
# speedup vs baseline: 1.0954x; 1.0160x over previous
"""Trainium2 Bass kernel for the Baller2Vec-style transformer encoder, v2.

Data parallel over batch (B=128) across 8 NeuronCores (16 per core, 8 pairs
of 2).  Feature-major activations.  The residual stream stores the
*unaffined* LayerNorm output (X-hat); each LN's per-feature gain/bias is
folded host-side into the adjacent weights and bias rows, removing three
DVE passes per LN.  QKV+V projections run as fp8(e4m3) DoubleRow matmuls
(weights hi+lo 2-term, activations hi) at 2x PE throughput; the rest is
bf16 with fp32 PSUM.  Sublayer biases enter PSUM via rank-1 matmuls;
softmax row-sum reciprocals are batched 3 heads per instruction; exp is
batched over 2-bank PSUM score tiles (4 segments per Activation op); FFN
relu runs on the otherwise idle Pool engine.
"""
import sys
import numpy as np

sys.path.insert(0, '/opt/trn_rl_repo')

import ml_dtypes
import concourse.bass as bass
import concourse.mybir as mybir
from concourse import tile
from concourse.bass_utils import run_bass_kernel_spmd
from concourse.vector_clock import ScopedClock

# ---------------------------------------------------------------- constants
HMAP = {0: (0, 0), 1: (0, 1), 2: (1, 0), 3: (1, 1), 4: (2, 0), 5: (2, 1),
        6: (0, 2), 7: (1, 2)}
BC_TILES = {0: [0], 1: [1], 2: [2], 3: [0, 1]}
P, T, E, D, H, F, L, V, C = 10, 20, 64, 512, 8, 2048, 6, 512, 9
B = 128
S = (P + 2) * T          # 240
HD = D // H
SCALE = float(np.sqrt(D))
HSCALE = 1.0 / float(np.sqrt(HD))
NCORES = 8
BPC = B // NCORES
NPAIR = BPC // 2
PC = 2 * S               # 480
NC_CH = D // 128
NF_CH = F // 128
TK = 120
EPS = 1e-5
SX = 16.0                # fp8 activation scale

bf16 = mybir.dt.bfloat16
f32 = mybir.dt.float32
fp8 = mybir.dt.float8e4
AF = mybir.ActivationFunctionType
ALU = mybir.AluOpType
PM = mybir.MatmulPerfMode
e4np = ml_dtypes.float8_e4m3fn


def _mask_np():
    m = np.full((S, S), -np.inf, dtype=np.float32)
    bs, cs = P * T, P * T + T
    for s in range(T):
        a, b_ = P * s, P * s + P
        m[a:b_, :b_] = 0.0
        m[a:b_, bs:bs + s + 1] = 0.0
        m[a:b_, cs:cs + s + 1] = 0.0
        for r in (bs + s, cs + s):
            m[r, :b_] = 0.0
            m[r, bs:bs + s + 1] = 0.0
            m[r, cs:cs + s + 1] = 0.0
    return m


# ---------------------------------------------------------------- tile patch
MAX_WAITS_PER_INST = 1


def _patched_drain_and_barrier(self, tick_clock, wait_clock):
    nc = self.nc
    probe = nc.sync.nop(nofuse=True)
    wait_clock.add_sem_waits(probe.ins, ScopedClock({None: tick_clock.global_clock}))
    si = probe.ins.sync_info
    waits = list(si.on_wait) if si is not None else []
    K = MAX_WAITS_PER_INST
    probe.ins.sync_info = mybir.SyncInfo(on_wait=waits[:K], on_update=[])
    for j in range(K, len(waits), K):
        w = nc.sync.nop(nofuse=True)
        w.ins.sync_info = mybir.SyncInfo(on_wait=waits[j:j + K], on_update=[])
    nc.sync.drain()
    nc.all_engine_barrier()
    popped = nc._tile_sem_poison_stack.pop()
    assert popped is self._sem_poison
    nc.clear_and_free_semaphores(list(self.sems.allocated().values()))
    nc.all_engine_barrier()


tile.TileContext._drain_and_barrier = _patched_drain_and_barrier


def _split_waits(nc, K=1):
    wsid = [0]
    for bb in nc.m.functions[0].blocks:
        out = []
        changed = False
        for inst in bb.instructions:
            si = inst.sync_info
            waits = list(si.on_wait) if si is not None else []
            if len(waits) > K:
                changed = True
                extra = waits[:-K]
                for j in range(0, len(extra), K):
                    nop = mybir.InstNoOp(name=f"WSNOP-{wsid[0]}")
                    wsid[0] += 1
                    nop.engine = inst.engine
                    nop.sync_info = mybir.SyncInfo(on_wait=extra[j:j + K],
                                                   on_update=[])
                    out.append(nop)
                inst.sync_info = mybir.SyncInfo(on_wait=waits[-K:],
                                                on_update=list(si.on_update))
            out.append(inst)
        if changed:
            bb.instructions = out


# ---------------------------------------------------------------- builder
def build(qdescale=(1.0,) * L, sx0=SX, n_layers=L, n_pairs=NPAIR):
    """qdescale[l]: immediate multiplier applied when copying the fp8
    QKV/V psum back to bf16 (1/(sx*sw)); qdescale[0] unused."""
    nc = bass.Bass("TRN2", target_bir_lowering=False)
    dt_in = {
        'pinT': ([E + 2, BPC * T * P], bf16),
        'ballT': ([E + 2, BPC * T], bf16),
        'maskb': ([TK, 2, S], bf16),
        'pw1': ([E + 2, 128], bf16),
        'pw2': ([128, NC_CH * 128], bf16),
        'bw1': ([E + 2, 128], bf16),
        'bw2': ([128, NC_CH * 128], bf16),
        'pb1': ([128, 1], f32),
        'pb2s': ([128, NC_CH], f32),
        'bb1': ([128, 1], f32),
        'bb2s': ([128, NC_CH], f32),
        'wq8': ([L, 128, 2, 2, 2, 3 * D], fp8),
        'wo': ([L, 128, NC_CH, D], bf16),
        'wf1': ([L, 128, NC_CH, F], bf16),
        'wf2': ([L, 128, NF_CH, D], bf16),
        'brows': ([L, 1, 2 * NC_CH, 128], bf16),
        'cvec': ([128, L, 32], f32),
        'selcf': ([96, 3, NC_CH * 128], bf16),
        'onesbf': ([128, 1], bf16),
        'onesd': ([128, 1], bf16),
        'onesrow': ([1, PC], bf16),
        'clsw': ([128, NC_CH, C], bf16),
        'clsb': ([C, 1], f32),
        'clse': ([128, NC_CH], f32),
    }
    dins = {k: nc.dram_tensor(k, sh, dt, kind="ExternalInput")
            for k, (sh, dt) in dt_in.items()}
    dout = nc.dram_tensor("out", [C, BPC], f32, kind="ExternalOutput")

    with tile.TileContext(nc) as tc:
        _body(nc, tc, dins, dout, qdescale, sx0, n_layers, n_pairs)
    _split_waits(nc)
    return nc


def _body(nc, tc, dins, dout, qdescale, sx0, n_layers, n_pairs):
    import contextlib
    ctx = contextlib.ExitStack()
    with ctx:
        persist = ctx.enter_context(tc.tile_pool(name="persist", bufs=1))
        X = persist.tile([128, NC_CH, BPC * S], bf16)     # residual (X-hat)
        XQ8 = persist.tile([128, NC_CH, BPC * S], fp8)    # fp8 shadow for qkv

        maskb = persist.tile([TK, 2, S], bf16)
        nc.sync.dma_start(maskb[:], dins['maskb'][:])
        cvec = persist.tile([128, L, 32], f32)
        nc.sync.dma_start(cvec[:], dins['cvec'][:])
        selcf = persist.tile([96, 3, NC_CH * 128], bf16)
        nc.sync.dma_start(selcf[:], dins['selcf'][:])
        onesbf = persist.tile([128, 1], bf16)
        nc.sync.dma_start(onesbf[:], dins['onesbf'][:])
        onesd = persist.tile([128, 1], bf16)
        nc.sync.dma_start(onesd[:], dins['onesd'][:])
        onesw = persist.tile([128, 32], bf16)
        nc.vector.memset(onesw[:], 1.0)
        onesrow = persist.tile([1, PC], bf16)
        nc.sync.dma_start(onesrow[:], dins['onesrow'][:])
        epsc = persist.tile([1, 1], f32)
        nc.gpsimd.memset(epsc[:], EPS)
        clsw = persist.tile([128, NC_CH, C], bf16)
        nc.sync.dma_start(clsw[:], dins['clsw'][:])
        clsb = persist.tile([C, 1], f32)
        nc.sync.dma_start(clsb[:], dins['clsb'][:])
        clse = persist.tile([128, NC_CH], f32)
        nc.sync.dma_start(clse[:], dins['clse'][:])

        # -------------------------------------------------- front end
        with tc.tile_pool(name="fe", bufs=1) as fe, \
             tc.tile_pool(name="feps", bufs=2, space="PSUM") as feps:
            pint = fe.tile([E + 2, BPC * T * P], bf16)
            nc.sync.dma_start(pint[:], dins['pinT'][:])
            ballT = fe.tile([E + 2, BPC * T], bf16)
            nc.sync.dma_start(ballT[:], dins['ballT'][:])
            pw1 = fe.tile([E + 2, 128], bf16)
            nc.sync.dma_start(pw1[:], dins['pw1'][:])
            pw2 = fe.tile([128, NC_CH * 128], bf16)
            nc.sync.dma_start(pw2[:], dins['pw2'][:])
            bw1 = fe.tile([E + 2, 128], bf16)
            nc.sync.dma_start(bw1[:], dins['bw1'][:])
            bw2 = fe.tile([128, NC_CH * 128], bf16)
            nc.sync.dma_start(bw2[:], dins['bw2'][:])
            pb1 = fe.tile([128, 1], f32)
            nc.sync.dma_start(pb1[:], dins['pb1'][:])
            pb2s = fe.tile([128, NC_CH], f32)
            nc.sync.dma_start(pb2s[:], dins['pb2s'][:])
            bb1 = fe.tile([128, 1], f32)
            nc.sync.dma_start(bb1[:], dins['bb1'][:])
            bb2s = fe.tile([128, NC_CH], f32)
            nc.sync.dma_start(bb2s[:], dins['bb2s'][:])

            for pi in range(NPAIR):
                h1p = feps.tile([128, 400], f32, tag="feps")
                nc.tensor.matmul(h1p[:], pw1[:], pint[:, pi * 400:(pi + 1) * 400],
                                 start=True, stop=True)
                h1b = fe.tile([128, 400], bf16, tag="h1b")
                nc.vector.tensor_scalar(h1b[:], h1p[:], pb1[:, 0:1], 0.0,
                                        ALU.add, ALU.max)
                for c in range(NC_CH):
                    pfp = feps.tile([128, 400], f32, tag="feps")
                    nc.tensor.matmul(pfp[:], pw2[:, c * 128:(c + 1) * 128], h1b[:],
                                     start=True, stop=True)
                    Xc = X[:, c, :].rearrange("p (j k) -> p j k",
                                              j=BPC * T, k=P + 2)
                    dst = Xc[:, 2 * pi * T:(2 * pi + 2) * T, 0:P]
                    nc.scalar.activation(dst, pfp[:].rearrange(
                        "p (j k) -> p j k", j=2 * T, k=P),
                        AF.Identity, bias=pb2s[:, c:c + 1], scale=SCALE)
            h1bl = feps.tile([128, 320], f32, tag="feps")
            nc.tensor.matmul(h1bl[:], bw1[:], ballT[:], start=True, stop=True)
            h1blb = fe.tile([128, 320], bf16, tag="h1b")
            nc.vector.tensor_scalar(h1blb[:], h1bl[:], bb1[:, 0:1], 0.0,
                                    ALU.add, ALU.max)
            for c in range(NC_CH):
                bfp = feps.tile([128, 320], f32, tag="feps")
                nc.tensor.matmul(bfp[:], bw2[:, c * 128:(c + 1) * 128], h1blb[:],
                                 start=True, stop=True)
                Xc = X[:, c, :].rearrange("p (j k) -> p j k", j=BPC * T, k=P + 2)
                src = bfp[:].rearrange("p (j k) -> p j k", j=BPC * T, k=1)
                nc.scalar.activation(Xc[:, :, P:P + 1], src,
                                     AF.Identity, bias=bb2s[:, c:c + 1], scale=SCALE)
                nc.scalar.activation(Xc[:, :, P + 1:P + 2], src,
                                     AF.Identity, bias=clse[:, c:c + 1], scale=0.0)
            nc.vector.tensor_scalar(XQ8[:, :, :], X[:, :, :], sx0, None,
                                    ALU.mult)

        # -------------------------------------------------- pipelined layers
        wpool = ctx.enter_context(tc.tile_pool(name="wq", bufs=2))
        wpool1 = ctx.enter_context(tc.tile_pool(name="wf", bufs=1))
        spool = ctx.enter_context(tc.tile_pool(name="scr", bufs=1))
        rpool = ctx.enter_context(tc.tile_pool(name="ring", bufs=2))
        spool2 = ctx.enter_context(tc.tile_pool(name="scr2", bufs=2))
        tpool = ctx.enter_context(tc.tile_pool(name="tmp", bufs=4))
        ups = ctx.enter_context(tc.tile_pool(name="ups", bufs=4, space="PSUM"))
        ln_ps = ctx.enter_context(tc.tile_pool(name="lnps", bufs=2, space="PSUM"))
        sc_ps = ctx.enter_context(tc.tile_pool(name="scps", bufs=2, space="PSUM"))

        W = {}

        def emit_A1(l, pi):
            """qkv + v projections for stream A."""
            pc = slice(pi * PC, (pi + 1) * PC)
            dsc = qdescale[l]
            qkb = spool.tile([128, 8, PC], bf16, tag="qkb")
            for j in range(8):
                ps = ups.tile([128, 512], f32, tag="u")
                n = 0
                for t in range(2):
                    for kk in range(2):
                        nc.tensor.matmul(
                            ps[:, :PC],
                            W['wq8'][:, kk, t, :, j * 128:(j + 1) * 128],
                            XQ8[:, 2 * kk:2 * kk + 2, pc],
                            start=(n == 0), stop=(n == 3),
                            perf_mode=PM.DoubleRow)
                        n += 1
                nc.scalar.activation(qkb[:, j, :], ps[:, :PC], AF.Identity,
                                     bias=cvec[:, l, j:j + 1], scale=dsc)
            vtm = spool.tile([128, 4, D], bf16, tag="vtm")
            for s4 in range(4):
                b, hf = divmod(s4, 2)
                ps = ups.tile([128, 512], f32, tag="u")
                glo = pi * PC + b * S + hf * TK
                n = 0
                for t in range(2):
                    for kk in range(2):
                        nc.tensor.matmul(
                            ps[:TK, :],
                            XQ8[:, 2 * kk:2 * kk + 2, glo:glo + TK],
                            W['wq8'][:, kk, t, :, 2 * D:3 * D],
                            start=(n == 0), stop=(n == 3),
                            perf_mode=PM.DoubleRow)
                        n += 1
                nc.scalar.activation(vtm[:TK, s4, :], ps[:TK, :], AF.Identity,
                                     scale=dsc)
            return {'qkb': qkb, 'vtm': vtm, 'pc': pc, 'l': l, 'pi': pi}

        def emit_A2(st_):
            """scores -> exp -> mask(Pool) -> rowsums -> recip."""
            qkb = st_['qkb']
            eT = rpool.tile([TK, 32, S], bf16, tag="big16")
            st_['eT'] = eT
            for h in range(H):
                hb = (h % 2) * 64
                jq, jk = h // 2, 4 + h // 2
                for b in range(2):
                    sp = sc_ps.tile([TK, 512], f32, tag="sc")
                    for s in range(2):
                        nc.tensor.matmul(
                            sp[:, s * S:(s + 1) * S],
                            qkb[hb:hb + 64, jk,
                                b * S + s * TK:b * S + s * TK + TK],
                            qkb[hb:hb + 64, jq, b * S:(b + 1) * S],
                            start=True, stop=True)
                    tmp = tpool.tile([TK, 2, S], bf16, tag="exp")
                    nc.scalar.activation(
                        tmp[:],
                        sp[:, 0:2 * S].rearrange("p (s n) -> p s n", s=2),
                        AF.Exp)
                    nc.vector.tensor_tensor(
                        eT[:, 4 * h + 2 * b:4 * h + 2 * b + 2, :],
                        tmp[:], maskb[:], ALU.mult)
            rsts = []
            for i in range(3):
                rsts.append(ups.tile([96, 512], f32, tag="u", name=f"rs{i}"))
            eTh = eT[:].rearrange("p (h x) n -> p h x n", h=H)
            for h in range(H):
                ti, sub = HMAP[h]
                base = sub * 32
                for s in range(2):
                    nc.tensor.matmul(
                        rsts[ti][base:base + 32, :PC],
                        onesw[:TK, :],
                        eTh[:, h, s:4:2, :],
                        start=(s == 0), stop=(s == 1))
            stg = spool.tile([96, 3, PC], bf16, tag="stg")
            with nc.allow_low_precision(reason="softmax recip"):
                for ti in range(3):
                    nc.vector.reciprocal(stg[:, ti, :], rsts[ti][:, :PC])
            st_['stg'] = stg

        def emit_A3(st_):
            """bc -> PV -> ofm ; Wo+bias ; residual STT ; LN1."""
            l, pc = st_['l'], st_['pc']
            eT, stg, vtm = st_['eT'], st_['stg'], st_['vtm']
            ofm = spool.tile([128, NC_CH, PC], bf16, tag="ofm")
            for c in range(NC_CH):
                bc = ups.tile([128, 512], f32, tag="u")
                tis = BC_TILES[c]
                for n, ti in enumerate(tis):
                    nc.tensor.matmul(bc[:, :PC],
                                     selcf[:, ti, c * 128:(c + 1) * 128],
                                     stg[:, ti, :],
                                     start=(n == 0), stop=(n == len(tis) - 1))
                bcs = spool.tile([128, PC], f32, tag="bcs")
                nc.scalar.activation(bcs[:], bc[:, :PC], AF.Identity)
                for b in range(2):
                    po = ups.tile([128, 512], f32, tag="u")
                    for hh in range(2):
                        h = 2 * c + hh
                        for s in range(2):
                            nc.tensor.matmul(
                                po[hh * 64:hh * 64 + 64, :S],
                                vtm[:TK, b * 2 + s, h * 64:(h + 1) * 64],
                                eT[:TK, 4 * h + 2 * b + s, :],
                                start=(s == 0), stop=(s == 1))
                    nc.vector.tensor_tensor(ofm[:, c, b * S:(b + 1) * S],
                                            bcs[:, b * S:(b + 1) * S],
                                            po[:, :S], ALU.mult)
            y = rpool.tile([128, NC_CH, PC], f32, tag="y")
            for c in range(NC_CH):
                ps = ups.tile([128, 512], f32, tag="u")
                nc.tensor.matmul(ps[:, :PC], W['brow'][0:1, c, :],
                                 onesrow[0:1, :], start=True, stop=False)
                for c2 in range(NC_CH):
                    nc.tensor.matmul(ps[:, :PC],
                                     W['wo'][:, c2, c * 128:(c + 1) * 128],
                                     ofm[:, c2, :],
                                     start=False, stop=(c2 == NC_CH - 1))
                nc.vector.scalar_tensor_tensor(y[:, c, :], X[:, c, pc],
                                               cvec[:, l, 8 + c:9 + c],
                                               ps[:, :PC], ALU.mult, ALU.add)
            _layernorm(nc, spool, rpool, ups, ln_ps, onesd, onesrow, epsc, y, X, pc)

        def emit_B1(l, pi):
            """FFN1 + relu (Pool)."""
            pc = slice(pi * PC, (pi + 1) * PC)
            hb_t = rpool.tile([128, NF_CH, PC], bf16, tag="big16")
            for fch in range(NF_CH):
                ps = ups.tile([128, 512], f32, tag="u")
                for c in range(NC_CH):
                    nc.tensor.matmul(ps[:, :PC],
                                     W['wf1'][:, c, fch * 128:(fch + 1) * 128],
                                     X[:, c, pc],
                                     start=(c == 0), stop=(c == NC_CH - 1))
                if fch % 2 == 0:
                    nc.scalar.activation(hb_t[:, fch, :], ps[:, :PC], AF.Relu,
                                         bias=cvec[:, l, 16 + fch:17 + fch])
                else:
                    nc.vector.tensor_scalar(hb_t[:, fch, :], ps[:, :PC],
                                            cvec[:, l, 16 + fch:17 + fch],
                                            0.0, ALU.add, ALU.max)
            return {'hb': hb_t, 'pc': pc, 'l': l, 'pi': pi}

        def emit_B2a(st_):
            """FFN2 + bias ; residual STT."""
            l, pc, hb_t = st_['l'], st_['pc'], st_['hb']
            y = rpool.tile([128, NC_CH, PC], f32, tag="y")
            st_['y'] = y
            for c in range(NC_CH):
                ps = ups.tile([128, 512], f32, tag="u")
                nc.tensor.matmul(ps[:, :PC], st_['brow'][0:1, NC_CH + c, :],
                                 onesrow[0:1, :], start=True, stop=False)
                for fch in range(NF_CH):
                    nc.tensor.matmul(ps[:, :PC],
                                     W['wf2'][:, fch, c * 128:(c + 1) * 128],
                                     hb_t[:, fch, :],
                                     start=False, stop=(fch == NF_CH - 1))
                nc.vector.scalar_tensor_tensor(y[:, c, :], X[:, c, pc],
                                               cvec[:, l, 12 + c:13 + c],
                                               ps[:, :PC], ALU.mult, ALU.add)

        def emit_B2b(st_, last_layer):
            """LN2 ; fp8 shadow cast."""
            pc = st_['pc']
            _layernorm(nc, spool, rpool, ups, ln_ps, onesd, onesrow, epsc,
                       st_['y'], X, pc)
            if not last_layer:
                nc.gpsimd.tensor_scalar(XQ8[:, :, pc], X[:, :, pc],
                                        SX, None, ALU.mult)

        nslots = n_layers * n_pairs
        stB = None
        for k in range(nslots + 1):
            newA = None
            if k < nslots:
                l, pi = divmod(k, n_pairs)
                if pi == 0:
                    W['wq8'] = wpool1.tile([128, 2, 2, 2, 3 * D], fp8,
                                           tag="wqf", name="wq8")
                    nc.sync.dma_start(W['wq8'][:], dins['wq8'][l])
                    W['wo'] = wpool.tile([128, NC_CH, D], bf16, tag="wo",
                                         name="wo")
                    nc.sync.dma_start(W['wo'][:], dins['wo'][l])
                    W['brow'] = wpool.tile([1, 2 * NC_CH, 128], bf16,
                                           tag="brow", name="brow")
                    nc.sync.dma_start(W['brow'][:], dins['brows'][l])
                if pi == min(1, n_pairs - 1):
                    W['wf1'] = wpool1.tile([128, NC_CH, F], bf16, tag="wf1",
                                           name="wf1")
                    nc.sync.dma_start(W['wf1'][:], dins['wf1'][l])
                    W['wf2'] = wpool1.tile([128, NF_CH, D], bf16, tag="wf2",
                                           name="wf2")
                    nc.sync.dma_start(W['wf2'][:], dins['wf2'][l])
                newA = emit_A1(l, pi)
            if stB is not None:
                stB = dict(stB, **emit_B1(stB['l'], stB['pi']))
            if newA is not None:
                emit_A2(newA)
            if stB is not None:
                emit_B2a(stB)
            if newA is not None:
                emit_A3(newA)
            if stB is not None:
                emit_B2b(stB, stB['l'] == n_layers - 1)
                stB = None
            if newA is not None:
                stB = {'l': newA['l'], 'pi': newA['pi'], 'brow': W['brow']}

        # -------------------------------------------------- classifier
        psc = ups.tile([C, 512], f32, tag="u")
        for c in range(NC_CH):
            nc.tensor.matmul(psc[:, :BPC], clsw[:, c, :],
                             X[:, c, :].rearrange("p (b t) -> p b t", b=BPC, t=S)
                             [:, :, S - 1],
                             start=(c == 0), stop=(c == NC_CH - 1))
        osb = spool.tile([C, BPC], f32, tag="osb")
        nc.scalar.activation(osb[:], psc[:, :BPC], AF.Identity, bias=clsb[:, 0:1])
        nc.sync.dma_start(dout[:], osb[:])


def _layernorm(nc, spool, rpool, ups, ln_ps, onesd, onesrow, epsc, y, X, pc):
    """X[:, :, pc] <- (y - mu) * rstd   (gain/bias folded into weights).
    onesd = 1/D column so the stats matmuls produce mu / m2 directly."""
    ybf = rpool.tile([128, NC_CH, PC], bf16, tag="ybf")
    for c in range(NC_CH):
        nc.scalar.activation(ybf[:, c, :], y[:, c, :], AF.Identity)
    ysq = rpool.tile([128, NC_CH, PC], bf16, tag="ysq")
    nc.vector.tensor_tensor(ysq[:], ybf[:], ybf[:], ALU.mult)
    psA = ups.tile([1, 512], f32, tag="u", name="psA")
    for c in range(NC_CH):
        nc.tensor.matmul(psA[0:1, :PC], onesd[:, 0:1], ybf[:, c, :],
                         start=(c == 0), stop=(c == NC_CH - 1))
    psB = ups.tile([1, 512], f32, tag="u", name="psB")
    for c in range(NC_CH):
        nc.tensor.matmul(psB[0:1, :PC], onesd[:, 0:1], ysq[:, c, :],
                         start=(c == 0), stop=(c == NC_CH - 1))
    st = rpool.tile([1, 3, PC], f32, tag="st")     # 0=mu 1=var/sd 2=musq
    nc.scalar.square(st[0:1, 2, :], psA[0:1, :PC])
    nc.vector.scalar_tensor_tensor(st[0:1, 1, :], psB[0:1, :PC], 0.0,
                                   st[0:1, 2, :], ALU.add, ALU.subtract)
    nc.scalar.activation(st[0:1, 1, :], st[0:1, 1, :], AF.Sqrt,
                         bias=epsc[0:1, 0:1])
    stbf = rpool.tile([1, 2, PC], bf16, tag="stbf")
    with nc.allow_low_precision(reason="ln rstd/mu broadcast"):
        nc.vector.reciprocal(stbf[0:1, 1, :], st[0:1, 1, :])
    nc.scalar.activation(stbf[0:1, 0, :], psA[0:1, :PC], AF.Identity)
    bcA = ln_ps.tile([128, PC], f32, tag="bc", name="bcA")
    nc.tensor.matmul(bcA[:], onesrow[0:1, 0:128], stbf[0:1, 1, :],
                     start=True, stop=True)
    bcB = ln_ps.tile([128, PC], f32, tag="bc", name="bcB")
    nc.tensor.matmul(bcB[:], onesrow[0:1, 0:128], stbf[0:1, 0, :],
                     start=True, stop=True)
    for c in range(NC_CH):
        nc.vector.tensor_tensor(y[:, c, :], y[:, c, :], bcB[:], ALU.subtract)
        nc.vector.tensor_tensor(X[:, c, pc], y[:, c, :], bcA[:], ALU.mult)


# ---------------------------------------------------------------- host side
_CACHED = {}


def _prep_consts(inputs):
    bf = ml_dtypes.bfloat16
    f32n = np.float32
    mask = _mask_np()
    maskbit = (mask == 0.0).astype(f32n)
    maskT = maskbit.T
    maskb = maskT.reshape(2, TK, S).transpose(1, 0, 2).astype(bf)

    def chunk_pm(vec, nch=NC_CH):
        return np.ascontiguousarray(vec.reshape(nch, 128).T)

    cons = {}
    cons['maskb'] = np.ascontiguousarray(maskb)
    cons['pw1'] = inputs['pW1'].astype(bf)
    cons['pw2'] = np.ascontiguousarray(
        inputs['pW2'].reshape(128, NC_CH * 128)).astype(bf)
    cons['bw1'] = inputs['bW1'].astype(bf)
    cons['bw2'] = np.ascontiguousarray(
        inputs['bW2'].reshape(128, NC_CH * 128)).astype(bf)
    cons['pb1'] = inputs['pb1'].reshape(128, 1).astype(f32n)
    cons['pb2s'] = (chunk_pm(inputs['pb2']) * SCALE).astype(f32n)
    cons['bb1'] = inputs['bb1'].reshape(128, 1).astype(f32n)
    cons['bb2s'] = (chunk_pm(inputs['bb2']) * SCALE).astype(f32n)

    g1 = inputs['ln1g']; b1 = inputs['ln1b']
    g2 = inputs['ln2g']; b2 = inputs['ln2b']

    # effective weights with LN gains folded in
    wq_eff = np.empty_like(inputs['Wqkv'])
    bq_eff = np.empty_like(inputs['bqkv'])
    for l in range(L):
        gin = np.ones(D, f32n) if l == 0 else g2[l - 1]
        bin_ = np.zeros(D, f32n) if l == 0 else b2[l - 1]
        w = inputs['Wqkv'][l] * gin[:, None]
        bq = inputs['bqkv'][l] + bin_ @ inputs['Wqkv'][l]
        w[:, :D] *= HSCALE
        bq[:D] *= HSCALE
        wq_eff[l] = w
        bq_eff[l] = bq

    # layer-0 activation scale from the actual front-end output range
    emb = inputs['emb'].astype(f32n)
    pe0 = emb[inputs['player_idxs'].astype(np.int64)]
    pin0 = np.concatenate([pe0, inputs['player_xs'][..., None],
                           inputs['player_ys'][..., None]], -1)
    pf0 = (np.maximum(pin0 @ inputs['pW1'] + inputs['pb1'], 0)
           @ inputs['pW2'] + inputs['pb2']) * SCALE
    be0 = np.broadcast_to(inputs['ball_e'], (B, T, E))
    bi0 = np.concatenate([be0, inputs['ball_xs'][..., None],
                          inputs['ball_ys'][..., None]], -1)
    bf0 = (np.maximum(bi0 @ inputs['bW1'] + inputs['bb1'], 0)
           @ inputs['bW2'] + inputs['bb2']) * SCALE
    amax0 = max(np.abs(pf0).max(), np.abs(bf0).max(),
                np.abs(inputs['cls_e']).max())
    sx0 = float(192.0 / max(amax0, 1e-9))

    qdescale = [1.0] * L
    wq8 = np.zeros((L, 128, 2, 2, 2, 3 * D), e4np)
    for l in range(0, L):
        w = wq_eff[l]
        sw = 192.0 / max(np.abs(w).max(), 1e-9)
        qdescale[l] = 1.0 / ((sx0 if l == 0 else SX) * sw)
        ws = (w * sw).astype(f32n)
        hi = ws.astype(e4np)
        lo = (ws - hi.astype(f32n)).astype(e4np)
        for kk in range(2):
            for i in range(2):
                ch = (2 * kk + i)
                wq8[l, :, kk, 0, i, :] = hi[ch * 128:(ch + 1) * 128, :]
                wq8[l, :, kk, 1, i, :] = lo[ch * 128:(ch + 1) * 128, :]
    cons['wq8'] = wq8

    def wlay(w, nch):
        Lw, K, N = w.shape
        return np.ascontiguousarray(
            w.reshape(Lw, nch, 128, N).transpose(0, 2, 1, 3)).astype(bf)

    wf1_eff = inputs['Wf1'] * g1[:, :, None]
    cons['wo'] = wlay(inputs['Wo'], NC_CH)
    cons['wf1'] = wlay(wf1_eff, NC_CH)
    cons['wf2'] = wlay(inputs['Wf2'], NF_CH)

    brows = np.zeros((L, 1, 2 * NC_CH, 128), f32n)
    cvec = np.zeros((128, L, 32), f32n)
    for l in range(L):
        bres = np.zeros(D, f32n) if l == 0 else b2[l - 1]
        gres = np.ones(D, f32n) if l == 0 else g2[l - 1]
        bo_eff = (inputs['bo'][l] + bq_eff[l][2 * D:] @ inputs['Wo'][l]
                  + bres)
        bf2_eff = inputs['bf2'][l] + b1[l]
        brows[l, 0, :NC_CH] = bo_eff.reshape(NC_CH, 128)
        brows[l, 0, NC_CH:] = bf2_eff.reshape(NC_CH, 128)
        cvec[:, l, 0:8] = np.ascontiguousarray(
            bq_eff[l][:2 * D].reshape(8, 128).T)
        cvec[:, l, 8:12] = chunk_pm(gres)
        cvec[:, l, 12:16] = chunk_pm(g1[l])
        cvec[:, l, 16:32] = np.ascontiguousarray(
            (inputs['bf1'][l] + b1[l] @ inputs['Wf1'][l]).reshape(NF_CH, 128).T)
    cons['brows'] = brows.astype(bf)
    cons['cvec'] = cvec

    selcf = np.zeros((96, 3, NC_CH * 128), f32n)
    for h in range(H):
        ti, sub = HMAP[h]
        c, half = divmod(h, 2)
        selcf[sub * 32, ti, c * 128 + half * 64: c * 128 + half * 64 + 64] = 1.0
    cons['selcf'] = selcf.astype(bf)
    cons['onesbf'] = np.ones((128, 1), bf)
    cons['onesd'] = np.full((128, 1), 1.0 / D, bf)
    cons['onesrow'] = np.ones((1, PC), bf)

    clsw_eff = inputs['clsW'] * g2[L - 1][:, None]
    clsb_eff = inputs['clsb'] + b2[L - 1] @ inputs['clsW']
    cons['clsw'] = np.ascontiguousarray(
        clsw_eff.reshape(NC_CH, 128, C).transpose(1, 0, 2)).astype(bf)
    cons['clsb'] = clsb_eff.reshape(C, 1).astype(f32n)
    cons['clse'] = chunk_pm(inputs['cls_e']).astype(f32n)
    return cons, qdescale, sx0


def kernel(**inputs):
    inputs = {k: np.asarray(v) for k, v in inputs.items()}
    bf = ml_dtypes.bfloat16
    cons, qdescale, sx0 = _prep_consts(inputs)
    if 'nc' not in _CACHED:
        _CACHED['nc'] = build(tuple(qdescale), sx0)
    nc = _CACHED['nc']

    emb = inputs['emb'].astype(np.float32)
    pe = emb[inputs['player_idxs'].astype(np.int64)]
    pin = np.concatenate([pe,
                          inputs['player_xs'][..., None],
                          inputs['player_ys'][..., None]], -1)
    ball_e = np.broadcast_to(inputs['ball_e'], (B, T, E))
    bi = np.concatenate([ball_e,
                         inputs['ball_xs'][..., None],
                         inputs['ball_ys'][..., None]], -1)

    in_maps = []
    for core in range(NCORES):
        bs = slice(core * BPC, (core + 1) * BPC)
        m = dict(cons)
        m['pinT'] = np.ascontiguousarray(
            pin[bs].reshape(BPC * T * P, E + 2).T).astype(bf)
        m['ballT'] = np.ascontiguousarray(
            bi[bs].reshape(BPC * T, E + 2).T).astype(bf)
        in_maps.append(m)

    res = run_bass_kernel_spmd(nc, in_maps, core_ids=list(range(NCORES)))
    outs = [res.results[c]['out'] for c in range(NCORES)]
    full = np.concatenate([o.T for o in outs], axis=0)
    return full.astype(np.float32)


if __name__ == "__main__":
    nc = build(n_layers=1, n_pairs=1)
    print("build ok")
